# revision 13
# baseline (speedup 1.0000x reference)
"""Encoder-layer Bass/Tile kernel for TRN2, data-parallel over 8 cores.

Layout strategy: feature-major ("transposed") activations throughout.
Core c handles batch b = c//2, sequence half h = c%2 (1024 query tokens).
Host rotates each core's x^T so that *its* tokens are always columns
0:1024 — the program is identical across cores (pure SPMD); attention is
permutation-invariant over keys so the rotated K/V order is harmless.

x and all weights are bf16 (full PE rate, halves SBUF + DMA); PSUM
accumulation, LayerNorm statistics and the final residual stream s2 are
fp32.  Measured end-to-end error vs the fp32 reference ~3e-3.

Per-core pipeline:
  P0/P1 attention: x streamed block-major so V/K/Q projection matmuls
    start as soon as block 0 lands.  Per head-pair: scores^T = K_h Q_h^T
    (two heads in PE row-groups), exp on ACT (scale=1/8, no
    max-subtraction), O^T accumulated over key tiles with a ones column
    per head (softmax row-sum trick).  The next pair's K/Q/V projection
    matmuls are *interleaved into the ki loop* (thunk pump) so the
    tensor queue never drains while ACT computes exp.  PSUM eviction on
    ACT (Copy); softmax normalization via reciprocal_approx_fast +
    gpsimd broadcast, deferred one qc-slot.
  P2 out-proj + residual + LN1, residual written in place over x^T's
    own-token columns; LN stats matmuls interleaved one d-tile behind
    the projection stream; LN1-qc0 finish overlaps qc1's matmuls,
    LN1-qc1 finish overlaps the first FFN1 quarter.
  P3 FFN in d_ff quarters (ReLU on ACT eviction); LN2 stats interleaved
    into the last quarter; qc0's normalize+store overlaps qc1's FFN2.
"""

import contextlib
from collections import deque

import numpy as np

import concourse.bass as bass  # noqa: F401
import concourse.mybir as mybir
import concourse.tile as tile
from concourse import bacc

N_CORES = 8
P = 128
D = 1024
DFF = 4096
H = 16
HD = 64
NPAIR = 8
T_MY = 1024
T_KV = 2048
DT = D // P
FT = DFF // P
KT = T_KV // P
EPS = 1e-5
B, S = 4, 2048

F32 = mybir.dt.float32
FR = mybir.dt.float32r
BF = mybir.dt.bfloat16
AF = mybir.ActivationFunctionType
ALU = mybir.AluOpType

DEFAULT_FLAGS = {
    "bq": False, "bk": False, "bv": False, "bo": False,
    "b1": False, "b2": False, "ln1": False, "ln2": False,
}


def build_encoder(flags=None, hw_reps=1, phases='all'):
    f = dict(DEFAULT_FLAGS)
    if flags:
        f.update(flags)

    nc = bacc.Bacc("TRN2", target_bir_lowering=False, debug=False,
                   num_devices=N_CORES)

    xT = nc.dram_tensor("xT", [D, T_KV], BF, kind="ExternalInput")
    # weights arrive host-pre-tiled (bf16) so every DMA is one contiguous
    # run per partition
    Wq = nc.dram_tensor("Wq", [NPAIR, P, DT, P], BF, kind="ExternalInput")
    Wk = nc.dram_tensor("Wk", [NPAIR, P, DT, P], BF, kind="ExternalInput")
    Wv = nc.dram_tensor("Wv", [4, P, DT, 256], BF, kind="ExternalInput")
    Wo = nc.dram_tensor("Wo", [DT, P, NPAIR, P], BF, kind="ExternalInput")
    W1 = nc.dram_tensor("W1", [FT, P, DT, P], BF, kind="ExternalInput")
    W2 = nc.dram_tensor("W2", [4, DT, P, FT // 4, P], BF,
                        kind="ExternalInput")
    bq = nc.dram_tensor("bq", [D], F32, kind="ExternalInput")
    bk = nc.dram_tensor("bk", [D], F32, kind="ExternalInput")
    bv = nc.dram_tensor("bv", [D], BF, kind="ExternalInput")
    bo = nc.dram_tensor("bo", [D], F32, kind="ExternalInput")
    b1 = nc.dram_tensor("b1", [DFF], F32, kind="ExternalInput")
    b2 = nc.dram_tensor("b2", [D], F32, kind="ExternalInput")
    g1 = nc.dram_tensor("g1", [D], F32, kind="ExternalInput")
    be1 = nc.dram_tensor("be1", [D], F32, kind="ExternalInput")
    g2 = nc.dram_tensor("g2", [D], F32, kind="ExternalInput")
    be2 = nc.dram_tensor("be2", [D], F32, kind="ExternalInput")
    outT = nc.dram_tensor("outT", [D, T_MY], F32, kind="ExternalOutput")

    tensors = dict(
        xT=xT, Wq=Wq, Wk=Wk, Wv=Wv, Wo=Wo, W1=W1, W2=W2, bq=bq, bk=bk,
        bv=bv, bo=bo, b1=b1, b2=b2, g1=g1, be1=be1, g2=g2, be2=be2,
        outT=outT)

    with tile.TileContext(nc) as tc:
        if hw_reps > 1:
            with tc.For_i(0, hw_reps, 1):
                _body(nc, tc, tensors, f, phases)
        else:
            _body(nc, tc, tensors, f, phases)
    nc.compile()
    return nc


def _body(nc, tc, t, f, phases='all'):
    xT, Wq, Wk, Wv, Wo, W1, W2 = (t[k] for k in
                                  ("xT", "Wq", "Wk", "Wv", "Wo", "W1", "W2"))
    bq, bk, bv, bo, b1, b2 = (t[k] for k in ("bq", "bk", "bv", "bo", "b1",
                                             "b2"))
    g1, be1, g2, be2 = (t[k] for k in ("g1", "be1", "g2", "be2"))
    outT = t["outT"]

    ctx = contextlib.ExitStack()
    with ctx:
        ctx.enter_context(nc.allow_low_precision(
            reason="bf16 weights/activations are intended; stats stay f32"))
        const = ctx.enter_context(tc.tile_pool(name="const", bufs=1))
        onesF = const.tile([P, P], F32)
        nc.vector.memset(onesF[:], 1.0)
        ones_rb = const.tile([1, P], BF)
        nc.vector.tensor_copy(ones_rb[:], onesF[0:1, :])
        eps_c = const.tile([1, 1], F32)
        nc.vector.memset(eps_c[:], EPS)
        # stats stationaries pre-scaled by 1/D so the matmul yields the
        # mean / mean-square directly (1/1024 is exact in bf16)
        invDF = const.tile([P, P], F32)
        nc.vector.memset(invDF[:], 1.0 / D)
        ones_cb = const.tile([P, 1], BF)
        nc.vector.tensor_copy(ones_cb[:], invDF[:, 0:1])
        ones_cf = const.tile([P, 1], FR)
        nc.vector.tensor_copy(ones_cf[:], invDF[:, 0:1])
        ones_rf = const.tile([1, P], F32)
        nc.vector.tensor_copy(ones_rf[:], onesF[0:1, :])

        def vec_tile(pool, name, src, n):
            tl = pool.tile([P, n], F32, name=name)
            nc.vector.dma_start(tl[:], src.rearrange("(t p) -> p t", p=P))
            return tl

        bq_sb = vec_tile(const, "bq_sb", bq, DT) if f["bq"] else None
        bk_sb = vec_tile(const, "bk_sb", bk, DT) if f["bk"] else None
        bo_sb = vec_tile(const, "bo_sb", bo, DT) if f["bo"] else None
        b1_sb = vec_tile(const, "b1_sb", b1, FT) if f["b1"] else None
        b2_sb = vec_tile(const, "b2_sb", b2, DT) if f["b2"] else None
        g1_sb = vec_tile(const, "g1_sb", g1, DT) if f["ln1"] else None
        be1_sb = vec_tile(const, "be1_sb", be1, DT) if f["ln1"] else None
        g2_sb = vec_tile(const, "g2_sb", g2, DT) if f["ln2"] else None
        be2_sb = vec_tile(const, "be2_sb", be2, DT) if f["ln2"] else None
        if f["bv"]:
            bv_sb = const.tile([1, D], BF)
            nc.vector.dma_start(bv_sb[:], bv[None, :])

        # small pools whose tiles span P2 -> P3
        finp = ctx.enter_context(tc.tile_pool(name="finp", bufs=2))
        bbp = ctx.enter_context(tc.tile_pool(name="bbp", bufs=2))
        rwp = ctx.enter_context(tc.tile_pool(name="rwp", bufs=1))
        rws = ctx.enter_context(tc.tile_pool(name="rws", bufs=1))

        dma_engs = [nc.sync, nc.scalar, nc.gpsimd]

        def recip(out_ap, in_ap):
            # reciprocal_approx_fast reads partition 0 only — callers must
            # pass base-partition-0 sources (~18 correct bits, 5x faster)
            nc.vector.reciprocal_approx_fast(out=out_ap, in_=in_ap)

        def ln_chain(st_s, st_q, ps_pool=None):
            """PSUM stats (already /D) -> broadcast mean/rstd [P,512].

            ps_pool set: broadcast via K=1 matmuls on the (idle) tensor
            engine into PSUM instead of gpsimd — used for the tail chain.
            """
            mean = rwp.tile([1, 512], F32, tag="mean")
            nc.vector.tensor_copy(mean[:], st_s[:])
            m2 = rws.tile([1, 512], F32, tag="m2")
            nc.vector.tensor_mul(m2[:], mean[:], mean[:])
            var = rws.tile([1, 512], F32, tag="var")
            nc.vector.tensor_sub(var[:], st_q[:], m2[:])
            sd = rws.tile([1, 512], F32, tag="sd")
            nc.scalar.activation(sd[:], var[:], AF.Sqrt, bias=eps_c[:])
            rstd = rwp.tile([1, 512], F32, tag="rstd")
            recip(rstd[:], sd[:])
            if ps_pool is not None:
                mean_b = ps_pool.tile([P, 512], F32, tag="bc")
                nc.tensor.matmul(mean_b[:], ones_rf[:], mean[:],
                                 start=True, stop=True)
                rstd_b = ps_pool.tile([P, 512], F32, tag="bc")
                nc.tensor.matmul(rstd_b[:], ones_rf[:], rstd[:],
                                 start=True, stop=True)
            else:
                mean_b = bbp.tile([P, 512], F32, tag="mb")
                nc.gpsimd.partition_broadcast(mean_b[:], mean[:])
                rstd_b = bbp.tile([P, 512], F32, tag="rb")
                nc.gpsimd.partition_broadcast(rstd_b[:], rstd[:])
            return mean_b, rstd_b

        m2s = contextlib.ExitStack()
        with m2s:
            xp = m2s.enter_context(tc.tile_pool(name="xp", bufs=1))
            x_sb = xp.tile([P, DT, T_KV], BF)
            obp = m2s.enter_context(tc.tile_pool(name="obp", bufs=1))
            osb_big = obp.tile([P, NPAIR, T_MY], BF, tag="ob",
                               name="osb_big")
            xTv = xT.rearrange("(kt p) t -> p kt t", p=P)

            att = contextlib.ExitStack()
            with att:
                vp = att.enter_context(tc.tile_pool(name="vp", bufs=2))
                wvp = att.enter_context(tc.tile_pool(name="wvp", bufs=2))
                wqkp = att.enter_context(tc.tile_pool(name="wqkp", bufs=2))
                qkp = att.enter_context(tc.tile_pool(name="qkp", bufs=2))
                etp = att.enter_context(tc.tile_pool(name="etp", bufs=2))
                gbp = att.enter_context(tc.tile_pool(name="gbp", bufs=2))
                nrm = att.enter_context(tc.tile_pool(name="nrm", bufs=2))
                ps_sc = att.enter_context(
                    tc.tile_pool(name="ps_sc", bufs=2, space="PSUM"))
                ps_o = att.enter_context(
                    tc.tile_pool(name="ps_o", bufs=2, space="PSUM"))
                ps_pr = att.enter_context(
                    tc.tile_pool(name="ps_pr", bufs=2, space="PSUM"))

                wv_t, v_t, kq_t = {}, {}, {}

                def v_dma(grp):
                    wv_sb = wvp.tile([P, DT, 256], BF, tag="wv",
                                     name="wv_sb")
                    nc.sync.dma_start(wv_sb[:, 0:DT // 2, :],
                                      Wv[grp][:, 0:DT // 2, :])
                    nc.scalar.dma_start(wv_sb[:, DT // 2:DT, :],
                                        Wv[grp][:, DT // 2:DT, :])
                    wv_t[grp] = wv_sb
                    v_t[grp] = vp.tile([P, KT, 2, 130], BF, tag="v",
                                       name="v_sb")

                def kq_dma(p):
                    wq_p = wqkp.tile([P, DT, P], BF, tag="wq", name="wq_p")
                    nc.gpsimd.dma_start(wq_p[:], Wq[p])
                    wk_p = wqkp.tile([P, DT, P], BF, tag="wk", name="wk_p")
                    nc.gpsimd.dma_start(wk_p[:], Wk[p])
                    kt_sb = qkp.tile([P, 4, 512], BF, tag="kt",
                                     name="kt_sb")
                    qt_sb = qkp.tile([P, 2, 512], BF, tag="qt",
                                     name="qt_sb")
                    kq_t[p] = (wq_p, wk_p, kt_sb, qt_sb)

                def v_ones(grp):
                    v_sb = v_t[grp]
                    ones_src = onesF[:, 0:32].rearrange(
                        "p (a b o) -> p a b o", a=KT, b=2)
                    nc.vector.tensor_copy(v_sb[:, :, :, 64:65], ones_src)
                    nc.vector.tensor_copy(v_sb[:, :, :, 129:130], ones_src)

                def v_step(grp, tt):
                    wv_sb, v_sb = wv_t[grp], v_t[grp]
                    ps = ps_pr.tile([P, 256], F32, tag="pr", name="psv")
                    for k in range(DT):
                        nc.tensor.matmul(
                            ps[:], x_sb[:, k, tt * P:(tt + 1) * P],
                            wv_sb[:, k, :],
                            start=(k == 0),
                            stop=(k == DT - 1 and not f["bv"]))
                    if f["bv"]:
                        nc.tensor.matmul(
                            ps[:], ones_rb[:],
                            bv_sb[:, grp * 256:(grp + 1) * 256],
                            start=False, stop=True)
                    dst = v_sb[:, tt, :, :].rearrange(
                        "p pr (hip j) -> p pr hip j", hip=2)[:, :, :, 0:64]
                    src = ps.rearrange("p (pr hip j) -> p pr hip j",
                                       pr=2, hip=2)
                    nc.vector.tensor_copy(dst, src)

                def kt_step(p, c):
                    _, wk_p, kt_sb, _ = kq_t[p]
                    ps = ps_pr.tile([P, 512], F32, tag="pr", name="psk")
                    for k in range(DT):
                        nc.tensor.matmul(
                            ps[:], wk_p[:, k, :],
                            x_sb[:, k, c * 512:(c + 1) * 512],
                            start=(k == 0), stop=(k == DT - 1))
                    if f["bk"]:
                        nc.vector.tensor_scalar_add(
                            kt_sb[:, c, :], ps[:], bk_sb[:, p:p + 1])
                    else:
                        nc.vector.tensor_copy(kt_sb[:, c, :], ps[:])

                def qt_step(p, c):
                    wq_p, _, _, qt_sb = kq_t[p]
                    ps = ps_pr.tile([P, 512], F32, tag="pr", name="psq")
                    for k in range(DT):
                        nc.tensor.matmul(
                            ps[:], wq_p[:, k, :],
                            x_sb[:, k, c * 512:(c + 1) * 512],
                            start=(k == 0), stop=(k == DT - 1))
                    if f["bq"]:
                        nc.vector.tensor_scalar_add(
                            qt_sb[:, c, :], ps[:], bq_sb[:, p:p + 1])
                    else:
                        nc.vector.tensor_copy(qt_sb[:, c, :], ps[:])

                def kq_thunks(p):
                    return ([lambda c=c: kt_step(p, c) for c in range(4)]
                            + [lambda c=c: qt_step(p, c) for c in range(2)])

                def v_thunks(grp):
                    out = [lambda: (v_ones(grp), v_step(grp, 0))]
                    out += [lambda tt=tt: v_step(grp, tt)
                            for tt in range(1, KT)]
                    return out

                # weight DMAs for pair 0 first, then x block-major
                v_dma(0)
                kq_dma(0)
                for blk in range(4):
                    for k in range(DT):
                        dma_engs[(blk * DT + k) % 3].dma_start(
                            x_sb[:, k, blk * 512:(blk + 1) * 512],
                            xTv[:, k, blk * 512:(blk + 1) * 512])

                # prologue: just enough to start pair-0 qc0
                v_ones(0)
                for tt in range(4):
                    v_step(0, tt)
                kt_step(0, 0)
                qt_step(0, 0)
                qt_step(0, 1)

                todo = deque()
                todo.extend([
                    (3, lambda: kt_step(0, 1)),
                    (4, lambda: v_step(0, 4)),
                    (5, lambda: v_step(0, 5)),
                    (6, lambda: v_step(0, 6)),
                    (7, lambda: kt_step(0, 2)),
                    (7, lambda: v_step(0, 7)),
                    (8, lambda: v_step(0, 8)),
                    (9, lambda: v_step(0, 9)),
                    (10, lambda: v_step(0, 10)),
                    (11, lambda: kt_step(0, 3)),
                    (11, lambda: v_step(0, 11)),
                    (12, lambda: v_step(0, 12)),
                    (13, lambda: v_step(0, 13)),
                    (14, lambda: v_step(0, 14)),
                    (15, lambda: v_step(0, 15)),
                ])

                pending_norm = deque()
                o_t = {}

                def emit_norm(p, qc):
                    oe, od = o_t.pop((p, qc))
                    qs = slice(qc * 512, (qc + 1) * 512)
                    den = nrm.tile([1, 1024], F32, tag="den")
                    nc.vector.tensor_copy(den[0:1, 0:512], oe[64:65, :])
                    nc.vector.tensor_copy(den[0:1, 512:1024], od[64:65, :])
                    rr = nrm.tile([1, 1024], F32, tag="rr")
                    recip(rr[:], den[:])
                    bc_e = nrm.tile([HD, 512], F32, tag="bce")
                    nc.gpsimd.partition_broadcast(bc_e[:], rr[0:1, 0:512])
                    bc_d = nrm.tile([HD, 512], F32, tag="bce")
                    nc.gpsimd.partition_broadcast(bc_d[:], rr[0:1, 512:1024])
                    nc.vector.tensor_mul(osb_big[0:HD, p, qs],
                                         oe[0:HD, :], bc_e[:])
                    nc.vector.tensor_mul(osb_big[HD:P, p, qs],
                                         od[0:HD, :], bc_d[:])

                for p in range(NPAIR):
                    grp, pl = p // 2, p % 2
                    if p + 1 < NPAIR:
                        kq_dma(p + 1)
                        todo.extend((-1, fn) for fn in kq_thunks(p + 1))
                        if (p + 1) % 2 == 0:
                            v_dma((p + 1) // 2)
                            todo.extend((-1, fn)
                                        for fn in v_thunks((p + 1) // 2))
                    _, _, kt_sb, qt_sb = kq_t[p]
                    v_sb = v_t[grp]
                    for qc in range(2):
                        qs = slice(qc * 512, (qc + 1) * 512)
                        ps_e = ps_o.tile([P, 512], F32, tag="o")
                        ps_d = ps_o.tile([P, 512], F32, tag="o")

                        def mk_sc(ki):
                            sc = ps_sc.tile([P, 1024], F32, tag="sc")
                            ks = slice((ki % 4) * P, (ki % 4) * P + P)
                            nc.tensor.matmul(
                                sc[:, 0:512], kt_sb[0:HD, ki // 4, ks],
                                qt_sb[0:HD, qc, :], start=True, stop=True)
                            nc.tensor.matmul(
                                sc[:, 512:1024], kt_sb[HD:P, ki // 4, ks],
                                qt_sb[HD:P, qc, :], start=True, stop=True)
                            return sc

                        def pump(slot, slots_left):
                            n = 0
                            while todo and todo[0][0] != -1 and \
                                    todo[0][0] <= slot:
                                todo.popleft()[1]()
                                n += 1
                            if n == 0 and todo and slots_left > 0 and \
                                    todo[0][0] == -1:
                                want = -(-len(todo) // slots_left)
                                for _ in range(want):
                                    if not todo or todo[0][0] != -1:
                                        break
                                    todo.popleft()[1]()

                        pump(0, 32 - qc * 16)
                        sc_next = mk_sc(0)
                        for ki in range(KT):
                            sc = sc_next
                            if ki + 1 < KT:
                                pump(ki + 1, 32 - qc * 16 - ki - 1)
                                sc_next = mk_sc(ki + 1)
                            et = etp.tile([P, 1024], BF, tag="et")
                            nc.scalar.activation(
                                et[:], sc[:], AF.Exp,
                                scale=float(1 / np.sqrt(HD)))
                            if ki == 2 and pending_norm:
                                pending_norm.popleft()()
                            nc.tensor.matmul(
                                ps_e[0:65, :], v_sb[:, ki, pl, 0:65],
                                et[:, 0:512],
                                start=(ki == 0), stop=(ki == KT - 1))
                            nc.tensor.matmul(
                                ps_d[0:65, :], v_sb[:, ki, pl, 65:130],
                                et[:, 512:1024],
                                start=(ki == 0), stop=(ki == KT - 1))
                        # evict unnormalized O (+den row) on ACT engine
                        oe = gbp.tile([65, 512], F32, tag="oe")
                        nc.scalar.activation(oe[:], ps_e[0:65, :], AF.Copy)
                        od = gbp.tile([65, 512], F32, tag="od")
                        nc.scalar.activation(od[:], ps_d[0:65, :], AF.Copy)
                        o_t[(p, qc)] = (oe, od)
                        if p == NPAIR - 1 and qc == 1:
                            while pending_norm:
                                pending_norm.popleft()()
                            emit_norm(p, qc)
                        else:
                            pending_norm.append(
                                lambda p=p, qc=qc: emit_norm(p, qc))
                while pending_norm:
                    pending_norm.popleft()()
                while todo:
                    todo.popleft()[1]()

            if phases == "att":
                nc.sync.dma_start(outT[0:P, 0:512],
                                  osb_big.bitcast(F32)[:, 0, 0:512])
                return

            # ======== P2: out-proj + residual + LN1 ========
            # Residual written in place over x_sb's own-token columns;
            # "s" below is x_sb[:, d, 0:1024].
            def ln1_finish_d(qc, d, mean_b, rstd_b):
                qs = slice(qc * 512, (qc + 1) * 512)
                nc.vector.tensor_sub(x_sb[:, d, qs], x_sb[:, d, qs],
                                     mean_b[:])
                if f["ln1"]:
                    tmp = finp.tile([P, 512], F32, tag="ftmp")
                    nc.vector.tensor_mul(tmp[:], x_sb[:, d, qs],
                                         rstd_b[:])
                    nc.vector.tensor_scalar(
                        x_sb[:, d, qs], tmp[:], g1_sb[:, d:d + 1],
                        be1_sb[:, d:d + 1], ALU.mult, ALU.add)
                else:
                    nc.vector.tensor_mul(x_sb[:, d, qs],
                                         x_sb[:, d, qs], rstd_b[:])

            chain1 = {}
            p2 = contextlib.ExitStack()
            with p2:
                wop = p2.enter_context(tc.tile_pool(name="wop", bufs=8))
                sqp = p2.enter_context(tc.tile_pool(name="sqp", bufs=2))
                ps_ac = p2.enter_context(
                    tc.tile_pool(name="ps_ac", bufs=3, space="PSUM"))
                ps_st = p2.enter_context(
                    tc.tile_pool(name="ps_st", bufs=2, space="PSUM"))

                wo_t = []
                for d in range(DT):
                    wt = wop.tile([P, DT, P], BF, tag="wo")
                    dma_engs[d % 3].dma_start(wt[:], Wo[d])
                    wo_t.append(wt)

                sq_t = {}
                for qc in range(2):
                    qs = slice(qc * 512, (qc + 1) * 512)
                    st_s = ps_st.tile([1, 512], F32, tag="st")
                    st_q = ps_st.tile([1, 512], F32, tag="st")

                    def st_mm(d, qs=qs, st_s=st_s, st_q=st_q):
                        nc.tensor.matmul(st_s[:], ones_cb[:],
                                         x_sb[:, d, qs],
                                         start=(d == 0),
                                         stop=(d == DT - 1))
                        nc.tensor.matmul(st_q[:], ones_cf[:], sq_t[d][:],
                                         start=(d == 0),
                                         stop=(d == DT - 1))

                    for d in range(DT):
                        ps = ps_ac.tile([P, 512], F32, tag="ac")
                        for pr in range(NPAIR):
                            nc.tensor.matmul(ps[:], wo_t[d][:, pr, :],
                                             osb_big[:, pr, qs],
                                             start=(pr == 0),
                                             stop=(pr == NPAIR - 1))
                        if f["bo"]:
                            tmp = finp.tile([P, 512], F32, tag="ftmp")
                            nc.vector.tensor_scalar_add(
                                tmp[:], ps[:], bo_sb[:, d:d + 1])
                            nc.vector.tensor_add(
                                x_sb[:, d, qs], tmp[:], x_sb[:, d, qs])
                        else:
                            nc.vector.tensor_add(x_sb[:, d, qs], ps[:],
                                                 x_sb[:, d, qs])
                        sq = sqp.tile([P, 512], FR, tag="sq")
                        nc.scalar.square(sq[:], x_sb[:, d, qs])
                        sq_t[d] = sq
                        if d >= 1:
                            st_mm(d - 1)
                        if qc == 1 and d >= 1:
                            ln1_finish_d(0, d - 1, *chain1[0])
                    st_mm(DT - 1)
                    chain1[qc] = ln_chain(st_s, st_q)
                    if qc == 1:
                        ln1_finish_d(0, DT - 1, *chain1[0])

            if phases == "p2":
                nc.sync.dma_start(outT[0:P, 0:512],
                                  x_sb.bitcast(F32)[:, 0, 0:512])
                return

            # ======== P3: FFN (x_sb[:, :, 0:1024] holds h) ========
            p3 = contextlib.ExitStack()
            with p3:
                w1p = p3.enter_context(tc.tile_pool(name="w1p", bufs=8))
                ffp = p3.enter_context(tc.tile_pool(name="ffp", bufs=2))
                sqp3 = p3.enter_context(tc.tile_pool(name="sqp3", bufs=2))
                w2p = p3.enter_context(tc.tile_pool(name="w2p", bufs=4))
                ps_ac3 = p3.enter_context(
                    tc.tile_pool(name="ps_ac3", bufs=3, space="PSUM"))
                ps_st3 = p3.enter_context(
                    tc.tile_pool(name="ps_st3", bufs=2, space="PSUM"))
                ps_bc = p3.enter_context(
                    tc.tile_pool(name="ps_bc", bufs=2, space="PSUM"))
                NQ = 4
                FQ = FT // NQ
                # s2 reuses osb_big's pool slot (osb dead after out-proj)
                s2 = obp.tile([P, DT, T_MY], FR, tag="ob", name="s2")

                def ffn1_step(fo, fo_l, qc, w1t):
                    qs = slice(qc * 512, (qc + 1) * 512)
                    ps = ps_ac3.tile([P, 512], F32, tag="ac")
                    for k in range(DT):
                        nc.tensor.matmul(ps[:], w1t[:, k, :],
                                         x_sb[:, k, qs],
                                         start=(k == 0),
                                         stop=(k == DT - 1))
                    nc.scalar.activation(
                        ff1q[:, fo_l, qs], ps[:], AF.Relu,
                        bias=(b1_sb[:, fo:fo + 1] if f["b1"] else 0.0))

                def ffn2_step(quarter, d, qc, w2t):
                    qs = slice(qc * 512, (qc + 1) * 512)
                    ps = ps_ac3.tile([P, 512], F32, tag="ac")
                    for k in range(FQ):
                        nc.tensor.matmul(ps[:], w2t[:, k, :],
                                         ff1q[:, k, qs],
                                         start=(k == 0),
                                         stop=(k == FQ - 1))
                    if quarter == 0:
                        if f["b2"]:
                            nc.vector.tensor_scalar_add(
                                s2[:, d, qs], ps[:], b2_sb[:, d:d + 1])
                            nc.vector.tensor_add(
                                s2[:, d, qs], s2[:, d, qs],
                                x_sb[:, d, qs])
                        else:
                            nc.vector.tensor_add(s2[:, d, qs], ps[:],
                                                 x_sb[:, d, qs])
                    else:
                        nc.vector.tensor_add(s2[:, d, qs],
                                             s2[:, d, qs], ps[:])

                def ln2_finish_d(qc, d, mean_b, rstd_b, eng=None):
                    eng = eng or nc.vector
                    qs = slice(qc * 512, (qc + 1) * 512)
                    eng.tensor_sub(s2[:, d, qs], s2[:, d, qs],
                                   mean_b[:])
                    if f["ln2"]:
                        tmp = finp.tile([P, 512], F32, tag="ftmp")
                        eng.tensor_mul(tmp[:], s2[:, d, qs],
                                       rstd_b[:])
                        eng.tensor_scalar(
                            s2[:, d, qs], tmp[:], g2_sb[:, d:d + 1],
                            be2_sb[:, d:d + 1], ALU.mult, ALU.add)
                    else:
                        eng.tensor_mul(s2[:, d, qs],
                                       s2[:, d, qs], rstd_b[:])
                    nc.sync.dma_start(outT[d * P:(d + 1) * P, qs],
                                      s2[:, d, qs].bitcast(F32))

                for quarter in range(NQ):
                    ff1q = ffp.tile([P, FQ, T_MY], BF, tag="ff1")
                    if quarter == 0:
                        # qc-major: qc0 matmuls overlap LN1-qc1 finish
                        w1ts = []
                        for fo_l in range(FQ):
                            w1t = w1p.tile([P, DT, P], BF, tag="w1")
                            dma_engs[fo_l % 3].dma_start(
                                w1t[:], W1[quarter * FQ + fo_l])
                            w1ts.append(w1t)
                        for qc in range(2):
                            for fo_l in range(FQ):
                                ffn1_step(quarter * FQ + fo_l, fo_l, qc,
                                          w1ts[fo_l])
                                if qc == 0:
                                    ln1_finish_d(1, fo_l, *chain1[1])
                    else:
                        for fo_l in range(FQ):
                            fo = quarter * FQ + fo_l
                            w1t = w1p.tile([P, DT, P], BF, tag="w1")
                            dma_engs[fo_l % 3].dma_start(w1t[:], W1[fo])
                            for qc in range(2):
                                ffn1_step(fo, fo_l, qc, w1t)
                    if quarter < NQ - 1:
                        for d in range(DT):
                            w2t = w2p.tile([P, FQ, P], BF, tag="w2")
                            dma_engs[d % 3].dma_start(w2t[:],
                                                      W2[quarter, d])
                            for qc in range(2):
                                ffn2_step(quarter, d, qc, w2t)
                    else:
                        chain2 = {}
                        for qc in range(2):
                            qs = slice(qc * 512, (qc + 1) * 512)
                            st_s = ps_st3.tile([1, 512], F32, tag="st")
                            st_q = ps_st3.tile([1, 512], F32, tag="st")
                            sq3_t = {}

                            def st3_mm(d, qs=qs, st_s=st_s, st_q=st_q,
                                       sq3_t=sq3_t):
                                nc.tensor.matmul(st_s[:], ones_cf[:],
                                                 s2[:, d, qs],
                                                 start=(d == 0),
                                                 stop=(d == DT - 1))
                                nc.tensor.matmul(st_q[:], ones_cf[:],
                                                 sq3_t[d][:],
                                                 start=(d == 0),
                                                 stop=(d == DT - 1))

                            for d in range(DT):
                                w2t = w2p.tile([P, FQ, P], BF, tag="w2")
                                dma_engs[d % 3].dma_start(
                                    w2t[:], W2[quarter, d])
                                ffn2_step(quarter, d, qc, w2t)
                                sq = sqp3.tile([P, 512], FR, tag="sq3")
                                nc.scalar.square(sq[:], s2[:, d, qs])
                                sq3_t[d] = sq
                                if d >= 1:
                                    st3_mm(d - 1)
                                if qc == 1 and d >= 1:
                                    ln2_finish_d(0, d - 1, *chain2[0])
                            st3_mm(DT - 1)
                            chain2[qc] = ln_chain(
                                st_s, st_q,
                                ps_pool=(ps_bc if qc == 1 else None))
                            if qc == 1:
                                ln2_finish_d(0, DT - 1, *chain2[0])
                        for d in range(DT):
                            ln2_finish_d(1, d, *chain2[1])


# ---------------- host-side helpers ----------------

def shard_inputs(inputs):
    import ml_dtypes
    bf16 = ml_dtypes.bfloat16
    x = np.asarray(inputs["x"], dtype=np.float32)
    shared = {k: np.ascontiguousarray(np.asarray(inputs[k], np.float32))
              for k in ("bq", "bk", "bo", "b1", "b2", "g1", "be1",
                        "g2", "be2")}
    shared["bv"] = np.ascontiguousarray(
        np.asarray(inputs["bv"], np.float32)).astype(bf16)
    Wq = np.asarray(inputs["Wq"], np.float32)
    Wk = np.asarray(inputs["Wk"], np.float32)
    Wv = np.asarray(inputs["Wv"], np.float32)
    Wo = np.asarray(inputs["Wo"], np.float32)
    W1 = np.asarray(inputs["W1"], np.float32)
    W2 = np.asarray(inputs["W2"], np.float32)
    FQ = FT // 4
    shared["Wq"] = np.ascontiguousarray(
        Wq.reshape(DT, P, NPAIR, P).transpose(2, 1, 0, 3)).astype(bf16)
    shared["Wk"] = np.ascontiguousarray(
        Wk.reshape(DT, P, NPAIR, P).transpose(2, 1, 0, 3)).astype(bf16)
    shared["Wv"] = np.ascontiguousarray(
        Wv.reshape(DT, P, 4, 256).transpose(2, 1, 0, 3)).astype(bf16)
    shared["Wo"] = np.ascontiguousarray(
        Wo.reshape(NPAIR, P, DT, P).transpose(2, 1, 0, 3)).astype(bf16)
    shared["W1"] = np.ascontiguousarray(
        W1.reshape(DT, P, FT, P).transpose(2, 1, 0, 3)).astype(bf16)
    shared["W2"] = np.ascontiguousarray(
        W2.reshape(4, FQ, P, DT, P).transpose(0, 3, 2, 1, 4)).astype(bf16)
    maps = []
    for c in range(N_CORES):
        b, h = c // 2, c % 2
        xTb = x[b].T
        roll = np.concatenate([xTb[:, h * T_MY:], xTb[:, :h * T_MY]], axis=1)
        m = {"xT": np.ascontiguousarray(roll).astype(bf16)}
        m.update(shared)
        maps.append(m)
    return maps


def unshard_output(results):
    out = np.empty((B, S, D), np.float32)
    for c in range(N_CORES):
        b, h = c // 2, c % 2
        out[b, h * T_MY:(h + 1) * T_MY, :] = results[c]["outT"].T
    return out


def flags_from_inputs(inputs):
    def nz(k):
        return bool(np.any(np.asarray(inputs[k])))

    return {
        "bq": nz("bq"), "bk": nz("bk"), "bv": nz("bv"), "bo": nz("bo"),
        "b1": nz("b1"), "b2": nz("b2"),
        "ln1": nz("be1") or not np.allclose(np.asarray(inputs["g1"]), 1.0),
        "ln2": nz("be2") or not np.allclose(np.asarray(inputs["g2"]), 1.0),
    }


# ---------------- SPMD runner ----------------


import time

import jax
from jax.sharding import Mesh, PartitionSpec
from jax.experimental.shard_map import shard_map

import concourse.bass2jax as b2j


class SpmdRunner:
    def __init__(self, nc, n_cores: int):
        b2j.install_neuronx_cc_hook()
        self.nc = nc
        self.n_cores = n_cores

        partition_name = (
            nc.partition_id_tensor.name if nc.partition_id_tensor else None
        )
        in_names, out_names, out_avals, zero_outs = [], [], [], []
        for alloc in nc.m.functions[0].allocations:
            if not isinstance(alloc, mybir.MemoryLocationSet):
                continue
            name = alloc.memorylocations[0].name
            if alloc.kind == "ExternalInput":
                if name != partition_name:
                    in_names.append(name)
            elif alloc.kind == "ExternalOutput":
                shape = tuple(alloc.tensor_shape)
                dtype = mybir.dt.np(alloc.dtype)
                out_names.append(name)
                out_avals.append(jax.core.ShapedArray(shape, dtype))
                zero_outs.append(np.zeros(shape, dtype))
        self.in_names, self.out_names = in_names, out_names
        self.out_avals = out_avals
        n_params, n_outs = len(in_names), len(out_names)
        self.n_params = n_params

        all_in_names = list(in_names) + list(out_names)
        if partition_name is not None:
            all_in_names.append(partition_name)

        def _body(*args):
            operands = list(args)
            if partition_name is not None:
                operands.append(b2j.partition_id_tensor())
            outs = b2j._bass_exec_p.bind(
                *operands,
                out_avals=tuple(out_avals),
                in_names=tuple(all_in_names),
                out_names=tuple(out_names),
                lowering_input_output_aliases=(),
                sim_require_finite=True,
                sim_require_nnan=True,
                nc=nc,
            )
            return tuple(outs)

        devices = jax.devices()[:n_cores]
        self.mesh = Mesh(np.asarray(devices), ("core",))
        in_specs = (PartitionSpec("core"),) * (n_params + n_outs)
        out_specs = (PartitionSpec("core"),) * n_outs
        # No donation: keeps zero-out buffers reusable across repeated calls.
        self.fn = jax.jit(
            shard_map(
                _body,
                mesh=self.mesh,
                in_specs=in_specs,
                out_specs=out_specs,
                check_rep=False,
            ),
            keep_unused=True,
        )
        self.zero_outs = zero_outs
        self._dev_zeros = None

    def put_inputs(self, in_maps: list[dict[str, np.ndarray]]):
        """Concat per-core inputs on axis 0 and move to device once."""
        concat = [
            np.concatenate(
                [np.asarray(in_maps[c][n]) for c in range(self.n_cores)], axis=0
            )
            for n in self.in_names
        ]
        sharding = jax.sharding.NamedSharding(self.mesh, PartitionSpec("core"))
        dev_in = [jax.device_put(a, sharding) for a in concat]
        if self._dev_zeros is None:
            self._dev_zeros = [
                jax.device_put(
                    np.zeros((self.n_cores * z.shape[0], *z.shape[1:]), z.dtype),
                    sharding,
                )
                for z in self.zero_outs
            ]
        return dev_in

    def run(self, dev_in):
        outs = self.fn(*dev_in, *self._dev_zeros)
        jax.block_until_ready(outs)
        return outs

    def run_numpy(self, in_maps):
        dev_in = self.put_inputs(in_maps)
        outs = self.run(dev_in)
        res = []
        for c in range(self.n_cores):
            d = {}
            for i, name in enumerate(self.out_names):
                full = np.asarray(outs[i])
                per = full.reshape(self.n_cores, *self.out_avals[i].shape)
                d[name] = per[c]
            res.append(d)
        return res

    def time_runs(self, dev_in, n=10, warmup=2):
        for _ in range(warmup):
            self.run(dev_in)
        times = []
        for _ in range(n):
            t0 = time.perf_counter()
            self.run(dev_in)
            times.append(time.perf_counter() - t0)
        return times


# ---------------- public entry point ----------------

_CACHE = {}


def _get_runner(flag_key, flags):
    if flag_key not in _CACHE:
        nc = build_encoder(flags)
        _CACHE[flag_key] = SpmdRunner(nc, N_CORES)
    return _CACHE[flag_key]


def kernel(**inputs):
    """Full-input encoder layer on 8 NeuronCores; returns [B, S, D] f32."""
    flags = flags_from_inputs(inputs)
    key = tuple(sorted(flags.items()))
    in_maps = shard_inputs(inputs)
    try:
        runner = _get_runner(key, flags)
        results = runner.run_numpy(in_maps)
    except Exception:
        # Device/mesh hiccup: reset backends and retry once from scratch.
        _CACHE.clear()
        try:
            jax.clear_caches()
        except Exception:
            pass
        try:
            jax.extend.backend.clear_backends()
        except Exception:
            pass
        runner = _get_runner(key, flags)
        results = runner.run_numpy(in_maps)
    return unshard_output(results)


# revision 15
# speedup vs baseline: 1.1036x; 1.1036x over previous
"""Encoder-layer Bass/Tile kernel for TRN2, data-parallel over 8 cores.

Layout strategy: feature-major ("transposed") activations throughout.
Core c handles batch b = c//2, sequence half h = c%2 (1024 query tokens).
Host rotates each core's x^T so that *its* tokens are always columns
0:1024 — the program is identical across cores (pure SPMD); attention is
permutation-invariant over keys so the rotated K/V order is harmless.

x and all weights are bf16 (full PE rate, halves SBUF + DMA); PSUM
accumulation, LayerNorm statistics and the final residual stream s2 are
fp32.  Measured end-to-end error vs the fp32 reference ~3e-3.

Per-core pipeline:
  P0/P1 attention: x streamed block-major so V/K/Q projection matmuls
    start as soon as block 0 lands.  Per head-pair: scores^T = K_h Q_h^T
    (two heads in PE row-groups), exp on ACT (scale=1/8, no
    max-subtraction), O^T accumulated over key tiles with a ones column
    per head (softmax row-sum trick).  The next pair's K/Q/V projection
    matmuls are *interleaved into the ki loop* (thunk pump) so the
    tensor queue never drains while ACT computes exp.  PSUM eviction on
    ACT (Copy); softmax normalization via reciprocal_approx_fast +
    gpsimd broadcast, deferred one qc-slot.
  P2 out-proj + residual + LN1, residual written in place over x^T's
    own-token columns; LN stats matmuls interleaved one d-tile behind
    the projection stream; LN1-qc0 finish overlaps qc1's matmuls,
    LN1-qc1 finish overlaps the first FFN1 quarter.
  P3 FFN in d_ff quarters (ReLU on ACT eviction); LN2 stats interleaved
    into the last quarter; qc0's normalize+store overlaps qc1's FFN2.
"""

import contextlib
from collections import deque

import numpy as np

import concourse.bass as bass  # noqa: F401
import concourse.mybir as mybir
import concourse.tile as tile
from concourse import bacc

N_CORES = 8
P = 128
D = 1024
DFF = 4096
H = 16
HD = 64
NPAIR = 8
T_MY = 1024
T_KV = 2048
DT = D // P
FT = DFF // P
KT = T_KV // P
EPS = 1e-5
B, S = 4, 2048

F32 = mybir.dt.float32
FR = mybir.dt.float32r
BF = mybir.dt.bfloat16
AF = mybir.ActivationFunctionType
ALU = mybir.AluOpType

DEFAULT_FLAGS = {
    "bq": False, "bk": False, "bv": False, "bo": False,
    "b1": False, "b2": False, "ln1": False, "ln2": False,
}


def build_encoder(flags=None, hw_reps=1, phases='all'):
    f = dict(DEFAULT_FLAGS)
    if flags:
        f.update(flags)

    nc = bacc.Bacc("TRN2", target_bir_lowering=False, debug=False,
                   num_devices=N_CORES)

    xT = nc.dram_tensor("xT", [D, T_KV], BF, kind="ExternalInput")
    # weights arrive host-pre-tiled (bf16) so every DMA is one contiguous
    # run per partition
    Wq = nc.dram_tensor("Wq", [NPAIR, P, DT, P], BF, kind="ExternalInput")
    Wk = nc.dram_tensor("Wk", [NPAIR, P, DT, P], BF, kind="ExternalInput")
    Wv = nc.dram_tensor("Wv", [4, P, DT, 256], BF, kind="ExternalInput")
    Wo = nc.dram_tensor("Wo", [DT, P, NPAIR, P], BF, kind="ExternalInput")
    W1 = nc.dram_tensor("W1", [FT, P, DT, P], BF, kind="ExternalInput")
    W2 = nc.dram_tensor("W2", [4, DT, P, FT // 4, P], BF,
                        kind="ExternalInput")
    bq = nc.dram_tensor("bq", [D], F32, kind="ExternalInput")
    bk = nc.dram_tensor("bk", [D], F32, kind="ExternalInput")
    bv = nc.dram_tensor("bv", [D], BF, kind="ExternalInput")
    bo = nc.dram_tensor("bo", [D], F32, kind="ExternalInput")
    b1 = nc.dram_tensor("b1", [DFF], F32, kind="ExternalInput")
    b2 = nc.dram_tensor("b2", [D], F32, kind="ExternalInput")
    g1 = nc.dram_tensor("g1", [D], F32, kind="ExternalInput")
    be1 = nc.dram_tensor("be1", [D], F32, kind="ExternalInput")
    g2 = nc.dram_tensor("g2", [D], F32, kind="ExternalInput")
    be2 = nc.dram_tensor("be2", [D], F32, kind="ExternalInput")
    outT = nc.dram_tensor("outT", [D, T_MY], F32, kind="ExternalOutput")

    tensors = dict(
        xT=xT, Wq=Wq, Wk=Wk, Wv=Wv, Wo=Wo, W1=W1, W2=W2, bq=bq, bk=bk,
        bv=bv, bo=bo, b1=b1, b2=b2, g1=g1, be1=be1, g2=g2, be2=be2,
        outT=outT)

    with tile.TileContext(nc) as tc:
        if hw_reps > 1:
            with tc.For_i(0, hw_reps, 1):
                _body(nc, tc, tensors, f, phases)
        else:
            _body(nc, tc, tensors, f, phases)
    nc.compile()
    return nc


def _body(nc, tc, t, f, phases='all'):
    xT, Wq, Wk, Wv, Wo, W1, W2 = (t[k] for k in
                                  ("xT", "Wq", "Wk", "Wv", "Wo", "W1", "W2"))
    bq, bk, bv, bo, b1, b2 = (t[k] for k in ("bq", "bk", "bv", "bo", "b1",
                                             "b2"))
    g1, be1, g2, be2 = (t[k] for k in ("g1", "be1", "g2", "be2"))
    outT = t["outT"]

    ctx = contextlib.ExitStack()
    with ctx:
        ctx.enter_context(nc.allow_low_precision(
            reason="bf16 weights/activations are intended; stats stay f32"))
        const = ctx.enter_context(tc.tile_pool(name="const", bufs=1))
        onesF = const.tile([P, P], F32)
        nc.vector.memset(onesF[:], 1.0)
        # stats stationaries pre-scaled by 1/D so the matmul yields the
        # mean / mean-square directly (1/1024 is exact in bf16)
        ones_cb = const.tile([P, 1], BF)
        nc.vector.tensor_scalar_mul(ones_cb[:], onesF[:, 0:1], 1.0 / D)
        ones_cf = const.tile([P, 1], FR)
        nc.vector.tensor_scalar_mul(ones_cf[:], onesF[:, 0:1], 1.0 / D)
        ones_rb = const.tile([1, P], BF)
        nc.vector.tensor_copy(ones_rb[:], onesF[0:1, :])
        eps_c = const.tile([1, 1], F32)
        nc.vector.memset(eps_c[:], EPS)

        def vec_tile(pool, name, src, n):
            tl = pool.tile([P, n], F32, name=name)
            nc.vector.dma_start(tl[:], src.rearrange("(t p) -> p t", p=P))
            return tl

        bq_sb = vec_tile(const, "bq_sb", bq, DT) if f["bq"] else None
        bk_sb = vec_tile(const, "bk_sb", bk, DT) if f["bk"] else None
        bo_sb = vec_tile(const, "bo_sb", bo, DT) if f["bo"] else None
        b1_sb = vec_tile(const, "b1_sb", b1, FT) if f["b1"] else None
        b2_sb = vec_tile(const, "b2_sb", b2, DT) if f["b2"] else None
        g1_sb = vec_tile(const, "g1_sb", g1, DT) if f["ln1"] else None
        be1_sb = vec_tile(const, "be1_sb", be1, DT) if f["ln1"] else None
        g2_sb = vec_tile(const, "g2_sb", g2, DT) if f["ln2"] else None
        be2_sb = vec_tile(const, "be2_sb", be2, DT) if f["ln2"] else None
        if f["bv"]:
            bv_sb = const.tile([1, D], BF)
            nc.vector.dma_start(bv_sb[:], bv[None, :])

        # small pools whose tiles span P2 -> P3
        finp = ctx.enter_context(tc.tile_pool(name="finp", bufs=2))
        bbp = ctx.enter_context(tc.tile_pool(name="bbp", bufs=2))
        rwp = ctx.enter_context(tc.tile_pool(name="rwp", bufs=1))
        rws = ctx.enter_context(tc.tile_pool(name="rws", bufs=1))

        dma_engs = [nc.sync, nc.scalar, nc.gpsimd]

        def recip(out_ap, in_ap):
            # reciprocal_approx_fast reads partition 0 only — callers must
            # pass base-partition-0 sources (~18 correct bits, 5x faster)
            nc.vector.reciprocal_approx_fast(out=out_ap, in_=in_ap)

        def ln_chain(st_s, st_q, ps_pool=None):
            """PSUM stats (already /D) -> broadcast mean/rstd [P,512].

            ps_pool set: broadcast via K=1 matmuls on the (idle) tensor
            engine into PSUM instead of gpsimd — used for the tail chain.
            """
            mean = rwp.tile([1, 512], F32, tag="mean")
            nc.vector.tensor_copy(mean[:], st_s[:])
            m2 = rws.tile([1, 512], F32, tag="m2")
            nc.vector.tensor_mul(m2[:], mean[:], mean[:])
            var = rws.tile([1, 512], F32, tag="var")
            nc.vector.tensor_sub(var[:], st_q[:], m2[:])
            sd = rws.tile([1, 512], F32, tag="sd")
            nc.scalar.activation(sd[:], var[:], AF.Sqrt, bias=eps_c[:])
            rstd = rwp.tile([1, 512], F32, tag="rstd")
            recip(rstd[:], sd[:])
            if ps_pool is not None:
                mean_b = ps_pool.tile([P, 512], F32, tag="bc")
                nc.tensor.matmul(mean_b[:], onesF[0:1, :], mean[:],
                                 start=True, stop=True)
                rstd_b = ps_pool.tile([P, 512], F32, tag="bc")
                nc.tensor.matmul(rstd_b[:], onesF[0:1, :], rstd[:],
                                 start=True, stop=True)
            else:
                mean_b = bbp.tile([P, 512], F32, tag="mb")
                nc.gpsimd.partition_broadcast(mean_b[:], mean[:])
                rstd_b = bbp.tile([P, 512], F32, tag="rb")
                nc.gpsimd.partition_broadcast(rstd_b[:], rstd[:])
            return mean_b, rstd_b

        m2s = contextlib.ExitStack()
        with m2s:
            xp = m2s.enter_context(tc.tile_pool(name="xp", bufs=1))
            x_sb = xp.tile([P, DT, T_KV], BF)
            obp = m2s.enter_context(tc.tile_pool(name="obp", bufs=1))
            osb_big = obp.tile([P, NPAIR, T_MY], BF, tag="ob",
                               name="osb_big")
            xTv = xT.rearrange("(kt p) t -> p kt t", p=P)

            att = contextlib.ExitStack()
            with att:
                vp = att.enter_context(tc.tile_pool(name="vp", bufs=2))
                wvp = att.enter_context(tc.tile_pool(name="wvp", bufs=2))
                wqkp = att.enter_context(tc.tile_pool(name="wqkp", bufs=2))
                qkp = att.enter_context(tc.tile_pool(name="qkp", bufs=2))
                etp = att.enter_context(tc.tile_pool(name="etp", bufs=2))
                gbp = att.enter_context(tc.tile_pool(name="gbp", bufs=2))
                nrm = att.enter_context(tc.tile_pool(name="nrm", bufs=2))
                ps_sc = att.enter_context(
                    tc.tile_pool(name="ps_sc", bufs=2, space="PSUM"))
                ps_o = att.enter_context(
                    tc.tile_pool(name="ps_o", bufs=2, space="PSUM"))
                ps_pr = att.enter_context(
                    tc.tile_pool(name="ps_pr", bufs=2, space="PSUM"))

                wv_t, v_t, kq_t = {}, {}, {}

                def v_dma(grp):
                    wv_sb = wvp.tile([P, DT, 256], BF, tag="wv",
                                     name="wv_sb")
                    nc.sync.dma_start(wv_sb[:, 0:DT // 2, :],
                                      Wv[grp][:, 0:DT // 2, :])
                    nc.scalar.dma_start(wv_sb[:, DT // 2:DT, :],
                                        Wv[grp][:, DT // 2:DT, :])
                    wv_t[grp] = wv_sb
                    v_t[grp] = vp.tile([P, KT, 2, 130], BF, tag="v",
                                       name="v_sb")

                def kq_dma(p):
                    wq_p = wqkp.tile([P, DT, P], BF, tag="wq", name="wq_p")
                    nc.gpsimd.dma_start(wq_p[:], Wq[p])
                    wk_p = wqkp.tile([P, DT, P], BF, tag="wk", name="wk_p")
                    nc.gpsimd.dma_start(wk_p[:], Wk[p])
                    kt_sb = qkp.tile([P, 4, 512], BF, tag="kt",
                                     name="kt_sb")
                    qt_sb = qkp.tile([P, 2, 512], BF, tag="qt",
                                     name="qt_sb")
                    kq_t[p] = (wq_p, wk_p, kt_sb, qt_sb)

                def v_ones(grp):
                    v_sb = v_t[grp]
                    ones_src = onesF[:, 0:32].rearrange(
                        "p (a b o) -> p a b o", a=KT, b=2)
                    nc.vector.tensor_copy(v_sb[:, :, :, 64:65], ones_src)
                    nc.vector.tensor_copy(v_sb[:, :, :, 129:130], ones_src)

                def v_step(grp, tt):
                    wv_sb, v_sb = wv_t[grp], v_t[grp]
                    ps = ps_pr.tile([P, 256], F32, tag="pr", name="psv")
                    for k in range(DT):
                        nc.tensor.matmul(
                            ps[:], x_sb[:, k, tt * P:(tt + 1) * P],
                            wv_sb[:, k, :],
                            start=(k == 0),
                            stop=(k == DT - 1 and not f["bv"]))
                    if f["bv"]:
                        nc.tensor.matmul(
                            ps[:], ones_rb[:],
                            bv_sb[:, grp * 256:(grp + 1) * 256],
                            start=False, stop=True)
                    dst = v_sb[:, tt, :, :].rearrange(
                        "p pr (hip j) -> p pr hip j", hip=2)[:, :, :, 0:64]
                    src = ps.rearrange("p (pr hip j) -> p pr hip j",
                                       pr=2, hip=2)
                    nc.vector.tensor_copy(dst, src)

                def kt_step(p, c):
                    _, wk_p, kt_sb, _ = kq_t[p]
                    ps = ps_pr.tile([P, 512], F32, tag="pr", name="psk")
                    for k in range(DT):
                        nc.tensor.matmul(
                            ps[:], wk_p[:, k, :],
                            x_sb[:, k, c * 512:(c + 1) * 512],
                            start=(k == 0), stop=(k == DT - 1))
                    if f["bk"]:
                        nc.vector.tensor_scalar_add(
                            kt_sb[:, c, :], ps[:], bk_sb[:, p:p + 1])
                    else:
                        nc.vector.tensor_copy(kt_sb[:, c, :], ps[:])

                def qt_step(p, c):
                    wq_p, _, _, qt_sb = kq_t[p]
                    ps = ps_pr.tile([P, 512], F32, tag="pr", name="psq")
                    for k in range(DT):
                        nc.tensor.matmul(
                            ps[:], wq_p[:, k, :],
                            x_sb[:, k, c * 512:(c + 1) * 512],
                            start=(k == 0), stop=(k == DT - 1))
                    if f["bq"]:
                        nc.vector.tensor_scalar_add(
                            qt_sb[:, c, :], ps[:], bq_sb[:, p:p + 1])
                    else:
                        nc.vector.tensor_copy(qt_sb[:, c, :], ps[:])

                def kq_thunks(p):
                    return ([lambda c=c: kt_step(p, c) for c in range(4)]
                            + [lambda c=c: qt_step(p, c) for c in range(2)])

                def v_thunks(grp):
                    out = [lambda: (v_ones(grp), v_step(grp, 0))]
                    out += [lambda tt=tt: v_step(grp, tt)
                            for tt in range(1, KT)]
                    return out

                # weight DMAs for pair 0 first, then x block-major
                v_dma(0)
                kq_dma(0)
                for blk in range(4):
                    for k in range(DT):
                        dma_engs[(blk * DT + k) % 3].dma_start(
                            x_sb[:, k, blk * 512:(blk + 1) * 512],
                            xTv[:, k, blk * 512:(blk + 1) * 512])

                # prologue: just enough to start pair-0 qc0
                v_ones(0)
                for tt in range(4):
                    v_step(0, tt)
                kt_step(0, 0)
                qt_step(0, 0)
                qt_step(0, 1)

                todo = deque()
                todo.extend([
                    (3, lambda: kt_step(0, 1)),
                    (4, lambda: v_step(0, 4)),
                    (5, lambda: v_step(0, 5)),
                    (6, lambda: v_step(0, 6)),
                    (7, lambda: kt_step(0, 2)),
                    (7, lambda: v_step(0, 7)),
                    (8, lambda: v_step(0, 8)),
                    (9, lambda: v_step(0, 9)),
                    (10, lambda: v_step(0, 10)),
                    (11, lambda: kt_step(0, 3)),
                    (11, lambda: v_step(0, 11)),
                    (12, lambda: v_step(0, 12)),
                    (13, lambda: v_step(0, 13)),
                    (14, lambda: v_step(0, 14)),
                    (15, lambda: v_step(0, 15)),
                ])

                pending_norm = deque()
                o_t = {}

                def emit_norm(p, qc):
                    oe, od = o_t.pop((p, qc))
                    qs = slice(qc * 512, (qc + 1) * 512)
                    den = nrm.tile([1, 1024], F32, tag="den")
                    nc.vector.tensor_copy(den[0:1, 0:512], oe[64:65, :])
                    nc.vector.tensor_copy(den[0:1, 512:1024], od[64:65, :])
                    rr = nrm.tile([1, 1024], F32, tag="rr")
                    recip(rr[:], den[:])
                    bc_e = nrm.tile([HD, 512], F32, tag="bce")
                    nc.gpsimd.partition_broadcast(bc_e[:], rr[0:1, 0:512])
                    bc_d = nrm.tile([HD, 512], F32, tag="bce")
                    nc.gpsimd.partition_broadcast(bc_d[:], rr[0:1, 512:1024])
                    nc.vector.tensor_mul(osb_big[0:HD, p, qs],
                                         oe[0:HD, :], bc_e[:])
                    nc.vector.tensor_mul(osb_big[HD:P, p, qs],
                                         od[0:HD, :], bc_d[:])

                for p in range(NPAIR):
                    grp, pl = p // 2, p % 2
                    if p + 1 < NPAIR:
                        kq_dma(p + 1)
                        todo.extend((-1, fn) for fn in kq_thunks(p + 1))
                        if (p + 1) % 2 == 0:
                            v_dma((p + 1) // 2)
                            todo.extend((-1, fn)
                                        for fn in v_thunks((p + 1) // 2))
                    _, _, kt_sb, qt_sb = kq_t[p]
                    v_sb = v_t[grp]
                    for qc in range(2):
                        qs = slice(qc * 512, (qc + 1) * 512)
                        ps_e = ps_o.tile([P, 512], F32, tag="o")
                        ps_d = ps_o.tile([P, 512], F32, tag="o")

                        def mk_sc(ki):
                            sc = ps_sc.tile([P, 1024], F32, tag="sc")
                            ks = slice((ki % 4) * P, (ki % 4) * P + P)
                            nc.tensor.matmul(
                                sc[:, 0:512], kt_sb[0:HD, ki // 4, ks],
                                qt_sb[0:HD, qc, :], start=True, stop=True)
                            nc.tensor.matmul(
                                sc[:, 512:1024], kt_sb[HD:P, ki // 4, ks],
                                qt_sb[HD:P, qc, :], start=True, stop=True)
                            return sc

                        def pump(slot, slots_left):
                            n = 0
                            while todo and todo[0][0] != -1 and \
                                    todo[0][0] <= slot:
                                todo.popleft()[1]()
                                n += 1
                            if n == 0 and todo and slots_left > 0 and \
                                    todo[0][0] == -1:
                                want = -(-len(todo) // slots_left)
                                for _ in range(want):
                                    if not todo or todo[0][0] != -1:
                                        break
                                    todo.popleft()[1]()

                        pump(0, 32 - qc * 16)
                        sc_next = mk_sc(0)
                        for ki in range(KT):
                            sc = sc_next
                            if ki + 1 < KT:
                                pump(ki + 1, 32 - qc * 16 - ki - 1)
                                sc_next = mk_sc(ki + 1)
                            et = etp.tile([P, 1024], BF, tag="et")
                            nc.scalar.activation(
                                et[:], sc[:], AF.Exp,
                                scale=float(1 / np.sqrt(HD)))
                            if ki == 2 and pending_norm:
                                pending_norm.popleft()()
                            nc.tensor.matmul(
                                ps_e[0:65, :], v_sb[:, ki, pl, 0:65],
                                et[:, 0:512],
                                start=(ki == 0), stop=(ki == KT - 1))
                            nc.tensor.matmul(
                                ps_d[0:65, :], v_sb[:, ki, pl, 65:130],
                                et[:, 512:1024],
                                start=(ki == 0), stop=(ki == KT - 1))
                        # evict unnormalized O (+den row) on ACT engine
                        oe = gbp.tile([65, 512], F32, tag="oe")
                        nc.scalar.activation(oe[:], ps_e[0:65, :], AF.Copy)
                        od = gbp.tile([65, 512], F32, tag="od")
                        nc.scalar.activation(od[:], ps_d[0:65, :], AF.Copy)
                        o_t[(p, qc)] = (oe, od)
                        if p == NPAIR - 1 and qc == 1:
                            while pending_norm:
                                pending_norm.popleft()()
                            emit_norm(p, qc)
                        else:
                            pending_norm.append(
                                lambda p=p, qc=qc: emit_norm(p, qc))
                while pending_norm:
                    pending_norm.popleft()()
                while todo:
                    todo.popleft()[1]()

            if phases == "att":
                nc.sync.dma_start(outT[0:P, 0:512],
                                  osb_big.bitcast(F32)[:, 0, 0:512])
                return

            # ======== P2: out-proj + residual + LN1 ========
            # Residual written in place over x_sb's own-token columns;
            # "s" below is x_sb[:, d, 0:1024].
            def ln1_finish_d(qc, d, mean_b, rstd_b):
                qs = slice(qc * 512, (qc + 1) * 512)
                nc.vector.tensor_sub(x_sb[:, d, qs], x_sb[:, d, qs],
                                     mean_b[:])
                if f["ln1"]:
                    tmp = finp.tile([P, 512], F32, tag="ftmp")
                    nc.vector.tensor_mul(tmp[:], x_sb[:, d, qs],
                                         rstd_b[:])
                    nc.vector.tensor_scalar(
                        x_sb[:, d, qs], tmp[:], g1_sb[:, d:d + 1],
                        be1_sb[:, d:d + 1], ALU.mult, ALU.add)
                else:
                    nc.vector.tensor_mul(x_sb[:, d, qs],
                                         x_sb[:, d, qs], rstd_b[:])

            chain1 = {}
            p2 = contextlib.ExitStack()
            with p2:
                wop = p2.enter_context(tc.tile_pool(name="wop", bufs=8))
                sqp = p2.enter_context(tc.tile_pool(name="sqp", bufs=2))
                ps_ac = p2.enter_context(
                    tc.tile_pool(name="ps_ac", bufs=3, space="PSUM"))
                ps_st = p2.enter_context(
                    tc.tile_pool(name="ps_st", bufs=2, space="PSUM"))

                wo_t = []
                for d in range(DT):
                    wt = wop.tile([P, DT, P], BF, tag="wo")
                    dma_engs[d % 3].dma_start(wt[:], Wo[d])
                    wo_t.append(wt)

                sq_t = {}
                for qc in range(2):
                    qs = slice(qc * 512, (qc + 1) * 512)
                    st_s = ps_st.tile([1, 512], F32, tag="st")
                    st_q = ps_st.tile([1, 512], F32, tag="st")

                    def st_mm(d, qs=qs, st_s=st_s, st_q=st_q):
                        nc.tensor.matmul(st_s[:], ones_cb[:],
                                         x_sb[:, d, qs],
                                         start=(d == 0),
                                         stop=(d == DT - 1))
                        nc.tensor.matmul(st_q[:], ones_cf[:], sq_t[d][:],
                                         start=(d == 0),
                                         stop=(d == DT - 1))

                    for d in range(DT):
                        ps = ps_ac.tile([P, 512], F32, tag="ac")
                        for pr in range(NPAIR):
                            nc.tensor.matmul(ps[:], wo_t[d][:, pr, :],
                                             osb_big[:, pr, qs],
                                             start=(pr == 0),
                                             stop=(pr == NPAIR - 1))
                        if f["bo"]:
                            tmp = finp.tile([P, 512], F32, tag="ftmp")
                            nc.vector.tensor_scalar_add(
                                tmp[:], ps[:], bo_sb[:, d:d + 1])
                            nc.vector.tensor_add(
                                x_sb[:, d, qs], tmp[:], x_sb[:, d, qs])
                        else:
                            nc.vector.tensor_add(x_sb[:, d, qs], ps[:],
                                                 x_sb[:, d, qs])
                        sq = sqp.tile([P, 512], FR, tag="sq")
                        nc.scalar.square(sq[:], x_sb[:, d, qs])
                        sq_t[d] = sq
                        if d >= 1:
                            st_mm(d - 1)
                        if qc == 1 and d >= 1:
                            ln1_finish_d(0, d - 1, *chain1[0])
                    st_mm(DT - 1)
                    chain1[qc] = ln_chain(st_s, st_q)
                    if qc == 1:
                        ln1_finish_d(0, DT - 1, *chain1[0])

            if phases == "p2":
                nc.sync.dma_start(outT[0:P, 0:512],
                                  x_sb.bitcast(F32)[:, 0, 0:512])
                return

            # ======== P3: FFN (x_sb[:, :, 0:1024] holds h) ========
            p3 = contextlib.ExitStack()
            with p3:
                w1p = p3.enter_context(tc.tile_pool(name="w1p", bufs=8))
                ffp = p3.enter_context(tc.tile_pool(name="ffp", bufs=2))
                sqp3 = p3.enter_context(tc.tile_pool(name="sqp3", bufs=2))
                w2p = p3.enter_context(tc.tile_pool(name="w2p", bufs=4))
                ps_ac3 = p3.enter_context(
                    tc.tile_pool(name="ps_ac3", bufs=3, space="PSUM"))
                ps_st3 = p3.enter_context(
                    tc.tile_pool(name="ps_st3", bufs=2, space="PSUM"))
                ps_bc = p3.enter_context(
                    tc.tile_pool(name="ps_bc", bufs=2, space="PSUM"))
                NQ = 4
                FQ = FT // NQ
                # s2 reuses osb_big's pool slot (osb dead after out-proj)
                s2 = obp.tile([P, DT, T_MY], FR, tag="ob", name="s2")

                def ffn1_step(fo, fo_l, qc, w1t):
                    qs = slice(qc * 512, (qc + 1) * 512)
                    ps = ps_ac3.tile([P, 512], F32, tag="ac")
                    for k in range(DT):
                        nc.tensor.matmul(ps[:], w1t[:, k, :],
                                         x_sb[:, k, qs],
                                         start=(k == 0),
                                         stop=(k == DT - 1))
                    nc.scalar.activation(
                        ff1q[:, fo_l, qs], ps[:], AF.Relu,
                        bias=(b1_sb[:, fo:fo + 1] if f["b1"] else 0.0))

                def ffn2_step(quarter, d, qc, w2t):
                    qs = slice(qc * 512, (qc + 1) * 512)
                    ps = ps_ac3.tile([P, 512], F32, tag="ac")
                    for k in range(FQ):
                        nc.tensor.matmul(ps[:], w2t[:, k, :],
                                         ff1q[:, k, qs],
                                         start=(k == 0),
                                         stop=(k == FQ - 1))
                    if quarter == 0:
                        if f["b2"]:
                            nc.vector.tensor_scalar_add(
                                s2[:, d, qs], ps[:], b2_sb[:, d:d + 1])
                            nc.vector.tensor_add(
                                s2[:, d, qs], s2[:, d, qs],
                                x_sb[:, d, qs])
                        else:
                            nc.vector.tensor_add(s2[:, d, qs], ps[:],
                                                 x_sb[:, d, qs])
                    else:
                        nc.vector.tensor_add(s2[:, d, qs],
                                             s2[:, d, qs], ps[:])

                def ln2_finish_d(qc, d, mean_b, rstd_b, eng=None):
                    eng = eng or nc.vector
                    qs = slice(qc * 512, (qc + 1) * 512)
                    eng.tensor_sub(s2[:, d, qs], s2[:, d, qs],
                                   mean_b[:])
                    if f["ln2"]:
                        tmp = finp.tile([P, 512], F32, tag="ftmp")
                        eng.tensor_mul(tmp[:], s2[:, d, qs],
                                       rstd_b[:])
                        eng.tensor_scalar(
                            s2[:, d, qs], tmp[:], g2_sb[:, d:d + 1],
                            be2_sb[:, d:d + 1], ALU.mult, ALU.add)
                    else:
                        eng.tensor_mul(s2[:, d, qs],
                                       s2[:, d, qs], rstd_b[:])
                    nc.sync.dma_start(outT[d * P:(d + 1) * P, qs],
                                      s2[:, d, qs].bitcast(F32))

                for quarter in range(NQ):
                    ff1q = ffp.tile([P, FQ, T_MY], BF, tag="ff1")
                    if quarter == 0:
                        # qc-major: qc0 matmuls overlap LN1-qc1 finish
                        w1ts = []
                        for fo_l in range(FQ):
                            w1t = w1p.tile([P, DT, P], BF, tag="w1")
                            dma_engs[fo_l % 3].dma_start(
                                w1t[:], W1[quarter * FQ + fo_l])
                            w1ts.append(w1t)
                        for qc in range(2):
                            for fo_l in range(FQ):
                                ffn1_step(quarter * FQ + fo_l, fo_l, qc,
                                          w1ts[fo_l])
                                if qc == 0:
                                    ln1_finish_d(1, fo_l, *chain1[1])
                    else:
                        for fo_l in range(FQ):
                            fo = quarter * FQ + fo_l
                            w1t = w1p.tile([P, DT, P], BF, tag="w1")
                            dma_engs[fo_l % 3].dma_start(w1t[:], W1[fo])
                            for qc in range(2):
                                ffn1_step(fo, fo_l, qc, w1t)
                    if quarter < NQ - 1:
                        for d in range(DT):
                            w2t = w2p.tile([P, FQ, P], BF, tag="w2")
                            dma_engs[d % 3].dma_start(w2t[:],
                                                      W2[quarter, d])
                            for qc in range(2):
                                ffn2_step(quarter, d, qc, w2t)
                    else:
                        chain2 = {}
                        for qc in range(2):
                            qs = slice(qc * 512, (qc + 1) * 512)
                            st_s = ps_st3.tile([1, 512], F32, tag="st")
                            st_q = ps_st3.tile([1, 512], F32, tag="st")
                            sq3_t = {}

                            def st3_mm(d, qs=qs, st_s=st_s, st_q=st_q,
                                       sq3_t=sq3_t):
                                nc.tensor.matmul(st_s[:], ones_cf[:],
                                                 s2[:, d, qs],
                                                 start=(d == 0),
                                                 stop=(d == DT - 1))
                                nc.tensor.matmul(st_q[:], ones_cf[:],
                                                 sq3_t[d][:],
                                                 start=(d == 0),
                                                 stop=(d == DT - 1))

                            for d in range(DT):
                                w2t = w2p.tile([P, FQ, P], BF, tag="w2")
                                dma_engs[d % 3].dma_start(
                                    w2t[:], W2[quarter, d])
                                ffn2_step(quarter, d, qc, w2t)
                                sq = sqp3.tile([P, 512], FR, tag="sq3")
                                nc.scalar.square(sq[:], s2[:, d, qs])
                                sq3_t[d] = sq
                                if d >= 1:
                                    st3_mm(d - 1)
                                if qc == 1 and d >= 1:
                                    ln2_finish_d(0, d - 1, *chain2[0])
                            st3_mm(DT - 1)
                            chain2[qc] = ln_chain(
                                st_s, st_q,
                                ps_pool=(ps_bc if qc == 1 else None))
                            if qc == 1:
                                ln2_finish_d(0, DT - 1, *chain2[0])
                        for d in range(DT):
                            ln2_finish_d(1, d, *chain2[1])


# ---------------- host-side helpers ----------------

def shard_inputs(inputs):
    import ml_dtypes
    bf16 = ml_dtypes.bfloat16
    x = np.asarray(inputs["x"], dtype=np.float32)
    shared = {k: np.ascontiguousarray(np.asarray(inputs[k], np.float32))
              for k in ("bq", "bk", "bo", "b1", "b2", "g1", "be1",
                        "g2", "be2")}
    shared["bv"] = np.ascontiguousarray(
        np.asarray(inputs["bv"], np.float32)).astype(bf16)
    Wq = np.asarray(inputs["Wq"], np.float32)
    Wk = np.asarray(inputs["Wk"], np.float32)
    Wv = np.asarray(inputs["Wv"], np.float32)
    Wo = np.asarray(inputs["Wo"], np.float32)
    W1 = np.asarray(inputs["W1"], np.float32)
    W2 = np.asarray(inputs["W2"], np.float32)
    FQ = FT // 4
    shared["Wq"] = np.ascontiguousarray(
        Wq.reshape(DT, P, NPAIR, P).transpose(2, 1, 0, 3)).astype(bf16)
    shared["Wk"] = np.ascontiguousarray(
        Wk.reshape(DT, P, NPAIR, P).transpose(2, 1, 0, 3)).astype(bf16)
    shared["Wv"] = np.ascontiguousarray(
        Wv.reshape(DT, P, 4, 256).transpose(2, 1, 0, 3)).astype(bf16)
    shared["Wo"] = np.ascontiguousarray(
        Wo.reshape(NPAIR, P, DT, P).transpose(2, 1, 0, 3)).astype(bf16)
    shared["W1"] = np.ascontiguousarray(
        W1.reshape(DT, P, FT, P).transpose(2, 1, 0, 3)).astype(bf16)
    shared["W2"] = np.ascontiguousarray(
        W2.reshape(4, FQ, P, DT, P).transpose(0, 3, 2, 1, 4)).astype(bf16)
    maps = []
    for c in range(N_CORES):
        b, h = c // 2, c % 2
        xTb = x[b].T
        roll = np.concatenate([xTb[:, h * T_MY:], xTb[:, :h * T_MY]], axis=1)
        m = {"xT": np.ascontiguousarray(roll).astype(bf16)}
        m.update(shared)
        maps.append(m)
    return maps


def unshard_output(results):
    out = np.empty((B, S, D), np.float32)
    for c in range(N_CORES):
        b, h = c // 2, c % 2
        out[b, h * T_MY:(h + 1) * T_MY, :] = results[c]["outT"].T
    return out


def flags_from_inputs(inputs):
    def nz(k):
        return bool(np.any(np.asarray(inputs[k])))

    return {
        "bq": nz("bq"), "bk": nz("bk"), "bv": nz("bv"), "bo": nz("bo"),
        "b1": nz("b1"), "b2": nz("b2"),
        "ln1": nz("be1") or not np.allclose(np.asarray(inputs["g1"]), 1.0),
        "ln2": nz("be2") or not np.allclose(np.asarray(inputs["g2"]), 1.0),
    }


# ---------------- SPMD runner ----------------


import time

import jax
from jax.sharding import Mesh, PartitionSpec
from jax.experimental.shard_map import shard_map

import concourse.bass2jax as b2j


class SpmdRunner:
    def __init__(self, nc, n_cores: int):
        b2j.install_neuronx_cc_hook()
        self.nc = nc
        self.n_cores = n_cores

        partition_name = (
            nc.partition_id_tensor.name if nc.partition_id_tensor else None
        )
        in_names, out_names, out_avals, zero_outs = [], [], [], []
        for alloc in nc.m.functions[0].allocations:
            if not isinstance(alloc, mybir.MemoryLocationSet):
                continue
            name = alloc.memorylocations[0].name
            if alloc.kind == "ExternalInput":
                if name != partition_name:
                    in_names.append(name)
            elif alloc.kind == "ExternalOutput":
                shape = tuple(alloc.tensor_shape)
                dtype = mybir.dt.np(alloc.dtype)
                out_names.append(name)
                out_avals.append(jax.core.ShapedArray(shape, dtype))
                zero_outs.append(np.zeros(shape, dtype))
        self.in_names, self.out_names = in_names, out_names
        self.out_avals = out_avals
        n_params, n_outs = len(in_names), len(out_names)
        self.n_params = n_params

        all_in_names = list(in_names) + list(out_names)
        if partition_name is not None:
            all_in_names.append(partition_name)

        def _body(*args):
            operands = list(args)
            if partition_name is not None:
                operands.append(b2j.partition_id_tensor())
            outs = b2j._bass_exec_p.bind(
                *operands,
                out_avals=tuple(out_avals),
                in_names=tuple(all_in_names),
                out_names=tuple(out_names),
                lowering_input_output_aliases=(),
                sim_require_finite=True,
                sim_require_nnan=True,
                nc=nc,
            )
            return tuple(outs)

        devices = jax.devices()[:n_cores]
        self.mesh = Mesh(np.asarray(devices), ("core",))
        in_specs = (PartitionSpec("core"),) * (n_params + n_outs)
        out_specs = (PartitionSpec("core"),) * n_outs
        # No donation: keeps zero-out buffers reusable across repeated calls.
        self.fn = jax.jit(
            shard_map(
                _body,
                mesh=self.mesh,
                in_specs=in_specs,
                out_specs=out_specs,
                check_rep=False,
            ),
            keep_unused=True,
        )
        self.zero_outs = zero_outs
        self._dev_zeros = None

    def put_inputs(self, in_maps: list[dict[str, np.ndarray]]):
        """Concat per-core inputs on axis 0 and move to device once."""
        concat = [
            np.concatenate(
                [np.asarray(in_maps[c][n]) for c in range(self.n_cores)], axis=0
            )
            for n in self.in_names
        ]
        sharding = jax.sharding.NamedSharding(self.mesh, PartitionSpec("core"))
        dev_in = [jax.device_put(a, sharding) for a in concat]
        if self._dev_zeros is None:
            self._dev_zeros = [
                jax.device_put(
                    np.zeros((self.n_cores * z.shape[0], *z.shape[1:]), z.dtype),
                    sharding,
                )
                for z in self.zero_outs
            ]
        return dev_in

    def run(self, dev_in):
        outs = self.fn(*dev_in, *self._dev_zeros)
        jax.block_until_ready(outs)
        return outs

    def run_numpy(self, in_maps):
        dev_in = self.put_inputs(in_maps)
        outs = self.run(dev_in)
        res = []
        for c in range(self.n_cores):
            d = {}
            for i, name in enumerate(self.out_names):
                full = np.asarray(outs[i])
                per = full.reshape(self.n_cores, *self.out_avals[i].shape)
                d[name] = per[c]
            res.append(d)
        return res

    def time_runs(self, dev_in, n=10, warmup=2):
        for _ in range(warmup):
            self.run(dev_in)
        times = []
        for _ in range(n):
            t0 = time.perf_counter()
            self.run(dev_in)
            times.append(time.perf_counter() - t0)
        return times


# ---------------- public entry point ----------------

_CACHE = {}


def _get_runner(flag_key, flags):
    if flag_key not in _CACHE:
        nc = build_encoder(flags)
        _CACHE[flag_key] = SpmdRunner(nc, N_CORES)
    return _CACHE[flag_key]


def kernel(**inputs):
    """Full-input encoder layer on 8 NeuronCores; returns [B, S, D] f32."""
    flags = flags_from_inputs(inputs)
    key = tuple(sorted(flags.items()))
    in_maps = shard_inputs(inputs)
    try:
        runner = _get_runner(key, flags)
        results = runner.run_numpy(in_maps)
    except Exception:
        # Device/mesh hiccup: reset backends and retry once from scratch.
        _CACHE.clear()
        try:
            jax.clear_caches()
        except Exception:
            pass
        try:
            jax.extend.backend.clear_backends()
        except Exception:
            pass
        runner = _get_runner(key, flags)
        results = runner.run_numpy(in_maps)
    return unshard_output(results)


# revision 16
# speedup vs baseline: 1.1136x; 1.0091x over previous
"""Encoder-layer Bass/Tile kernel for TRN2, data-parallel over 8 cores.

Layout strategy: feature-major ("transposed") activations throughout.
Core c handles batch b = c//2, sequence half h = c%2 (1024 query tokens).
Host rotates each core's x^T so that *its* tokens are always columns
0:1024 — the program is identical across cores (pure SPMD); attention is
permutation-invariant over keys so the rotated K/V order is harmless.

x and all weights are bf16 (full PE rate, halves SBUF + DMA); PSUM
accumulation, LayerNorm statistics and the final residual stream s2 are
fp32.  Measured end-to-end error vs the fp32 reference ~3e-3.

Per-core pipeline:
  P0/P1 attention: x streamed block-major so V/K/Q projection matmuls
    start as soon as block 0 lands.  Per head-pair: scores^T = K_h Q_h^T
    (two heads in PE row-groups), exp on ACT (scale=1/8, no
    max-subtraction), O^T accumulated over key tiles with a ones column
    per head (softmax row-sum trick).  The next pair's K/Q/V projection
    matmuls are *interleaved into the ki loop* (thunk pump) so the
    tensor queue never drains while ACT computes exp.  PSUM eviction on
    ACT (Copy); softmax normalization via reciprocal_approx_fast +
    gpsimd broadcast, deferred one qc-slot.
  P2 out-proj + residual + LN1, residual written in place over x^T's
    own-token columns; LN stats matmuls interleaved one d-tile behind
    the projection stream; LN1-qc0 finish overlaps qc1's matmuls,
    LN1-qc1 finish overlaps the first FFN1 quarter.
  P3 FFN in d_ff quarters (ReLU on ACT eviction); LN2 stats interleaved
    into the last quarter; qc0's normalize+store overlaps qc1's FFN2.
"""

import contextlib
from collections import deque

import numpy as np

import concourse.bass as bass  # noqa: F401
import concourse.mybir as mybir
import concourse.tile as tile
from concourse import bacc

N_CORES = 8
P = 128
D = 1024
DFF = 4096
H = 16
HD = 64
NPAIR = 8
T_MY = 1024
T_KV = 2048
DT = D // P
FT = DFF // P
KT = T_KV // P
EPS = 1e-5
B, S = 4, 2048

F32 = mybir.dt.float32
FR = mybir.dt.float32r
BF = mybir.dt.bfloat16
AF = mybir.ActivationFunctionType
ALU = mybir.AluOpType

DEFAULT_FLAGS = {
    "bq": False, "bk": False, "bv": False, "bo": False,
    "b1": False, "b2": False, "ln1": False, "ln2": False,
}


def build_encoder(flags=None, hw_reps=1, phases='all'):
    f = dict(DEFAULT_FLAGS)
    if flags:
        f.update(flags)

    nc = bacc.Bacc("TRN2", target_bir_lowering=False, debug=False,
                   num_devices=N_CORES)

    xT = nc.dram_tensor("xT", [D, T_KV], BF, kind="ExternalInput")
    # weights arrive host-pre-tiled (bf16) so every DMA is one contiguous
    # run per partition
    Wq = nc.dram_tensor("Wq", [NPAIR, P, DT, P], BF, kind="ExternalInput")
    Wk = nc.dram_tensor("Wk", [NPAIR, P, DT, P], BF, kind="ExternalInput")
    Wv = nc.dram_tensor("Wv", [4, P, DT, 256], BF, kind="ExternalInput")
    Wo = nc.dram_tensor("Wo", [DT, P, NPAIR, P], BF, kind="ExternalInput")
    W1 = nc.dram_tensor("W1", [FT, P, DT, P], BF, kind="ExternalInput")
    W2 = nc.dram_tensor("W2", [4, DT, P, FT // 4, P], BF,
                        kind="ExternalInput")
    bq = nc.dram_tensor("bq", [D], F32, kind="ExternalInput")
    bk = nc.dram_tensor("bk", [D], F32, kind="ExternalInput")
    bv = nc.dram_tensor("bv", [D], BF, kind="ExternalInput")
    bo = nc.dram_tensor("bo", [D], F32, kind="ExternalInput")
    b1 = nc.dram_tensor("b1", [DFF], F32, kind="ExternalInput")
    b2 = nc.dram_tensor("b2", [D], F32, kind="ExternalInput")
    g1 = nc.dram_tensor("g1", [D], F32, kind="ExternalInput")
    be1 = nc.dram_tensor("be1", [D], F32, kind="ExternalInput")
    g2 = nc.dram_tensor("g2", [D], F32, kind="ExternalInput")
    be2 = nc.dram_tensor("be2", [D], F32, kind="ExternalInput")
    outT = nc.dram_tensor("outT", [D, T_MY], F32, kind="ExternalOutput")

    tensors = dict(
        xT=xT, Wq=Wq, Wk=Wk, Wv=Wv, Wo=Wo, W1=W1, W2=W2, bq=bq, bk=bk,
        bv=bv, bo=bo, b1=b1, b2=b2, g1=g1, be1=be1, g2=g2, be2=be2,
        outT=outT)

    with tile.TileContext(nc) as tc:
        if hw_reps > 1:
            with tc.For_i(0, hw_reps, 1):
                _body(nc, tc, tensors, f, phases)
        else:
            _body(nc, tc, tensors, f, phases)
    nc.compile()
    return nc


def _body(nc, tc, t, f, phases='all'):
    xT, Wq, Wk, Wv, Wo, W1, W2 = (t[k] for k in
                                  ("xT", "Wq", "Wk", "Wv", "Wo", "W1", "W2"))
    bq, bk, bv, bo, b1, b2 = (t[k] for k in ("bq", "bk", "bv", "bo", "b1",
                                             "b2"))
    g1, be1, g2, be2 = (t[k] for k in ("g1", "be1", "g2", "be2"))
    outT = t["outT"]

    ctx = contextlib.ExitStack()
    with ctx:
        ctx.enter_context(nc.allow_low_precision(
            reason="bf16 weights/activations are intended; stats stay f32"))
        const = ctx.enter_context(tc.tile_pool(name="const", bufs=1))
        onesF = const.tile([P, P], F32)
        nc.vector.memset(onesF[:], 1.0)
        # stats stationaries pre-scaled by 1/D so the matmul yields the
        # mean / mean-square directly (1/1024 is exact in bf16)
        ones_cb = const.tile([P, 1], BF)
        nc.vector.tensor_scalar_mul(ones_cb[:], onesF[:, 0:1], 1.0 / D)
        ones_cf = const.tile([P, 1], FR)
        nc.vector.tensor_scalar_mul(ones_cf[:], onesF[:, 0:1], 1.0 / D)
        ones_rb = const.tile([1, P], BF)
        nc.vector.tensor_copy(ones_rb[:], onesF[0:1, :])
        eps_c = const.tile([1, 1], F32)
        nc.vector.memset(eps_c[:], EPS)

        def vec_tile(pool, name, src, n):
            tl = pool.tile([P, n], F32, name=name)
            nc.vector.dma_start(tl[:], src.rearrange("(t p) -> p t", p=P))
            return tl

        bq_sb = vec_tile(const, "bq_sb", bq, DT) if f["bq"] else None
        bk_sb = vec_tile(const, "bk_sb", bk, DT) if f["bk"] else None
        bo_sb = vec_tile(const, "bo_sb", bo, DT) if f["bo"] else None
        b1_sb = vec_tile(const, "b1_sb", b1, FT) if f["b1"] else None
        b2_sb = vec_tile(const, "b2_sb", b2, DT) if f["b2"] else None
        g1_sb = vec_tile(const, "g1_sb", g1, DT) if f["ln1"] else None
        be1_sb = vec_tile(const, "be1_sb", be1, DT) if f["ln1"] else None
        g2_sb = vec_tile(const, "g2_sb", g2, DT) if f["ln2"] else None
        be2_sb = vec_tile(const, "be2_sb", be2, DT) if f["ln2"] else None
        if f["bv"]:
            bv_sb = const.tile([1, D], BF)
            nc.vector.dma_start(bv_sb[:], bv[None, :])

        # small pools whose tiles span P2 -> P3
        finp = ctx.enter_context(tc.tile_pool(name="finp", bufs=2))
        bbp = ctx.enter_context(tc.tile_pool(name="bbp", bufs=2))
        rwp = ctx.enter_context(tc.tile_pool(name="rwp", bufs=1))
        rws = ctx.enter_context(tc.tile_pool(name="rws", bufs=1))

        dma_engs = [nc.sync, nc.scalar, nc.gpsimd]

        def recip(out_ap, in_ap):
            # reciprocal_approx_fast reads partition 0 only — callers must
            # pass base-partition-0 sources (~18 correct bits, 5x faster)
            nc.vector.reciprocal_approx_fast(out=out_ap, in_=in_ap)

        def ln_chain(st_s, st_q, ps_pool=None):
            """PSUM stats (already /D) -> broadcast mean/rstd [P,512].

            ps_pool set: broadcast via K=1 matmuls on the (idle) tensor
            engine into PSUM instead of gpsimd — used for the tail chain.
            """
            mean = rwp.tile([1, 512], F32, tag="mean")
            nc.vector.tensor_copy(mean[:], st_s[:])
            m2 = rws.tile([1, 512], F32, tag="m2")
            nc.vector.tensor_mul(m2[:], mean[:], mean[:])
            var = rws.tile([1, 512], F32, tag="var")
            nc.vector.tensor_sub(var[:], st_q[:], m2[:])
            sd = rws.tile([1, 512], F32, tag="sd")
            nc.scalar.activation(sd[:], var[:], AF.Sqrt, bias=eps_c[:])
            rstd = rwp.tile([1, 512], F32, tag="rstd")
            recip(rstd[:], sd[:])
            if ps_pool is not None:
                mean_b = ps_pool.tile([P, 512], F32, tag="bc")
                nc.tensor.matmul(mean_b[:], onesF[0:1, :], mean[:],
                                 start=True, stop=True)
                rstd_b = ps_pool.tile([P, 512], F32, tag="bc")
                nc.tensor.matmul(rstd_b[:], onesF[0:1, :], rstd[:],
                                 start=True, stop=True)
            else:
                mean_b = bbp.tile([P, 512], F32, tag="mb")
                nc.gpsimd.partition_broadcast(mean_b[:], mean[:])
                rstd_b = bbp.tile([P, 512], F32, tag="rb")
                nc.gpsimd.partition_broadcast(rstd_b[:], rstd[:])
            return mean_b, rstd_b

        m2s = contextlib.ExitStack()
        with m2s:
            xp = m2s.enter_context(tc.tile_pool(name="xp", bufs=1))
            x_sb = xp.tile([P, DT, T_KV], BF)
            obp = m2s.enter_context(tc.tile_pool(name="obp", bufs=1))
            osb_big = obp.tile([P, NPAIR, T_MY], BF, tag="ob",
                               name="osb_big")
            xTv = xT.rearrange("(kt p) t -> p kt t", p=P)

            att = contextlib.ExitStack()
            with att:
                vp = att.enter_context(tc.tile_pool(name="vp", bufs=2))
                wvp = att.enter_context(tc.tile_pool(name="wvp", bufs=2))
                wqkp = att.enter_context(tc.tile_pool(name="wqkp", bufs=2))
                qkp = att.enter_context(tc.tile_pool(name="qkp", bufs=2))
                etp = att.enter_context(tc.tile_pool(name="etp", bufs=2))
                gbp = att.enter_context(tc.tile_pool(name="gbp", bufs=2))
                nrm = att.enter_context(tc.tile_pool(name="nrm", bufs=2))
                ps_sc = att.enter_context(
                    tc.tile_pool(name="ps_sc", bufs=2, space="PSUM"))
                ps_o = att.enter_context(
                    tc.tile_pool(name="ps_o", bufs=2, space="PSUM"))
                ps_pr = att.enter_context(
                    tc.tile_pool(name="ps_pr", bufs=2, space="PSUM"))

                wv_t, v_t, kq_t = {}, {}, {}

                def v_dma(grp):
                    wv_sb = wvp.tile([P, DT, 256], BF, tag="wv",
                                     name="wv_sb")
                    nc.sync.dma_start(wv_sb[:, 0:DT // 2, :],
                                      Wv[grp][:, 0:DT // 2, :])
                    nc.scalar.dma_start(wv_sb[:, DT // 2:DT, :],
                                        Wv[grp][:, DT // 2:DT, :])
                    wv_t[grp] = wv_sb
                    v_t[grp] = vp.tile([P, KT, 2, 130], BF, tag="v",
                                       name="v_sb")

                def kq_dma(p):
                    wq_p = wqkp.tile([P, DT, P], BF, tag="wq", name="wq_p")
                    nc.gpsimd.dma_start(wq_p[:], Wq[p])
                    wk_p = wqkp.tile([P, DT, P], BF, tag="wk", name="wk_p")
                    nc.gpsimd.dma_start(wk_p[:], Wk[p])
                    kt_sb = qkp.tile([P, 4, 512], BF, tag="kt",
                                     name="kt_sb")
                    qt_sb = qkp.tile([P, 2, 512], BF, tag="qt",
                                     name="qt_sb")
                    kq_t[p] = (wq_p, wk_p, kt_sb, qt_sb)

                def v_ones(grp):
                    v_sb = v_t[grp]
                    ones_src = onesF[:, 0:32].rearrange(
                        "p (a b o) -> p a b o", a=KT, b=2)
                    nc.vector.tensor_copy(v_sb[:, :, :, 64:65], ones_src)
                    nc.vector.tensor_copy(v_sb[:, :, :, 129:130], ones_src)

                def v_step(grp, tt):
                    wv_sb, v_sb = wv_t[grp], v_t[grp]
                    ps = ps_pr.tile([P, 256], F32, tag="pr", name="psv")
                    for k in range(DT):
                        nc.tensor.matmul(
                            ps[:], x_sb[:, k, tt * P:(tt + 1) * P],
                            wv_sb[:, k, :],
                            start=(k == 0),
                            stop=(k == DT - 1 and not f["bv"]))
                    if f["bv"]:
                        nc.tensor.matmul(
                            ps[:], ones_rb[:],
                            bv_sb[:, grp * 256:(grp + 1) * 256],
                            start=False, stop=True)
                    dst = v_sb[:, tt, :, :].rearrange(
                        "p pr (hip j) -> p pr hip j", hip=2)[:, :, :, 0:64]
                    src = ps.rearrange("p (pr hip j) -> p pr hip j",
                                       pr=2, hip=2)
                    nc.vector.tensor_copy(dst, src)

                def kt_step(p, c):
                    _, wk_p, kt_sb, _ = kq_t[p]
                    ps = ps_pr.tile([P, 512], F32, tag="pr", name="psk")
                    for k in range(DT):
                        nc.tensor.matmul(
                            ps[:], wk_p[:, k, :],
                            x_sb[:, k, c * 512:(c + 1) * 512],
                            start=(k == 0), stop=(k == DT - 1))
                    if f["bk"]:
                        nc.vector.tensor_scalar_add(
                            kt_sb[:, c, :], ps[:], bk_sb[:, p:p + 1])
                    else:
                        nc.vector.tensor_copy(kt_sb[:, c, :], ps[:])

                def qt_step(p, c):
                    wq_p, _, _, qt_sb = kq_t[p]
                    ps = ps_pr.tile([P, 512], F32, tag="pr", name="psq")
                    for k in range(DT):
                        nc.tensor.matmul(
                            ps[:], wq_p[:, k, :],
                            x_sb[:, k, c * 512:(c + 1) * 512],
                            start=(k == 0), stop=(k == DT - 1))
                    if f["bq"]:
                        nc.vector.tensor_scalar_add(
                            qt_sb[:, c, :], ps[:], bq_sb[:, p:p + 1])
                    else:
                        nc.vector.tensor_copy(qt_sb[:, c, :], ps[:])

                def kq_thunks(p):
                    return ([lambda c=c: kt_step(p, c) for c in range(4)]
                            + [lambda c=c: qt_step(p, c) for c in range(2)])

                def v_thunks(grp):
                    out = [lambda: (v_ones(grp), v_step(grp, 0))]
                    out += [lambda tt=tt: v_step(grp, tt)
                            for tt in range(1, KT)]
                    return out

                # weight DMAs for pair 0 first, then x block-major
                v_dma(0)
                kq_dma(0)
                for blk in range(4):
                    for k in range(DT):
                        dma_engs[(blk * DT + k) % 3].dma_start(
                            x_sb[:, k, blk * 512:(blk + 1) * 512],
                            xTv[:, k, blk * 512:(blk + 1) * 512])

                # prologue: just enough to start pair-0 qc0
                v_ones(0)
                for tt in range(4):
                    v_step(0, tt)
                kt_step(0, 0)
                qt_step(0, 0)
                qt_step(0, 1)

                todo = deque()
                todo.extend([
                    (3, lambda: kt_step(0, 1)),
                    (4, lambda: v_step(0, 4)),
                    (5, lambda: v_step(0, 5)),
                    (6, lambda: v_step(0, 6)),
                    (7, lambda: kt_step(0, 2)),
                    (7, lambda: v_step(0, 7)),
                    (8, lambda: v_step(0, 8)),
                    (9, lambda: v_step(0, 9)),
                    (10, lambda: v_step(0, 10)),
                    (11, lambda: kt_step(0, 3)),
                    (11, lambda: v_step(0, 11)),
                    (12, lambda: v_step(0, 12)),
                    (13, lambda: v_step(0, 13)),
                    (14, lambda: v_step(0, 14)),
                    (15, lambda: v_step(0, 15)),
                ])

                pending_norm = deque()
                o_t = {}

                def emit_norm(p, qc):
                    oe, od = o_t.pop((p, qc))
                    qs = slice(qc * 512, (qc + 1) * 512)
                    den = nrm.tile([1, 1024], F32, tag="den")
                    nc.vector.tensor_copy(den[0:1, 0:512], oe[64:65, :])
                    nc.vector.tensor_copy(den[0:1, 512:1024], od[64:65, :])
                    rr = nrm.tile([1, 1024], F32, tag="rr")
                    recip(rr[:], den[:])
                    bc_e = nrm.tile([HD, 512], F32, tag="bce")
                    nc.gpsimd.partition_broadcast(bc_e[:], rr[0:1, 0:512])
                    bc_d = nrm.tile([HD, 512], F32, tag="bce")
                    nc.gpsimd.partition_broadcast(bc_d[:], rr[0:1, 512:1024])
                    nc.vector.tensor_mul(osb_big[0:HD, p, qs],
                                         oe[0:HD, :], bc_e[:])
                    nc.vector.tensor_mul(osb_big[HD:P, p, qs],
                                         od[0:HD, :], bc_d[:])

                for p in range(NPAIR):
                    grp, pl = p // 2, p % 2
                    if p + 1 < NPAIR:
                        kq_dma(p + 1)
                        if p + 1 == NPAIR - 1:
                            # spread pair-7's projections into the
                            # otherwise-starved pair-6/7 exp bubbles
                            todo.extend([
                                (208, lambda: kt_step(7, 0)),
                                (214, lambda: qt_step(7, 0)),
                                (227, lambda: kt_step(7, 1)),
                                (231, lambda: kt_step(7, 2)),
                                (235, lambda: kt_step(7, 3)),
                                (238, lambda: qt_step(7, 1)),
                            ])
                        else:
                            todo.extend((-1, fn)
                                        for fn in kq_thunks(p + 1))
                        if (p + 1) % 2 == 0:
                            g = (p + 1) // 2
                            v_dma(g)
                            if g == 3:
                                todo.extend((-1, fn)
                                            for fn in v_thunks(3)[0:8])
                                todo.extend(
                                    (192 + tt,
                                     lambda tt=tt: v_step(3, tt))
                                    for tt in range(8, KT))
                            else:
                                todo.extend((-1, fn)
                                            for fn in v_thunks(g))
                    _, _, kt_sb, qt_sb = kq_t[p]
                    v_sb = v_t[grp]
                    for qc in range(2):
                        qs = slice(qc * 512, (qc + 1) * 512)
                        ps_e = ps_o.tile([P, 512], F32, tag="o")
                        ps_d = ps_o.tile([P, 512], F32, tag="o")

                        def mk_sc(ki):
                            sc = ps_sc.tile([P, 1024], F32, tag="sc")
                            ks = slice((ki % 4) * P, (ki % 4) * P + P)
                            nc.tensor.matmul(
                                sc[:, 0:512], kt_sb[0:HD, ki // 4, ks],
                                qt_sb[0:HD, qc, :], start=True, stop=True)
                            nc.tensor.matmul(
                                sc[:, 512:1024], kt_sb[HD:P, ki // 4, ks],
                                qt_sb[HD:P, qc, :], start=True, stop=True)
                            return sc

                        def pump(slot, slots_left):
                            n = 0
                            while todo and todo[0][0] != -1 and \
                                    todo[0][0] <= slot:
                                todo.popleft()[1]()
                                n += 1
                            if n == 0 and todo and slots_left > 0 and \
                                    todo[0][0] == -1:
                                want = -(-len(todo) // slots_left)
                                for _ in range(want):
                                    if not todo or todo[0][0] != -1:
                                        break
                                    todo.popleft()[1]()

                        base = (p * 2 + qc) * 16
                        pump(base, 32 - qc * 16)
                        sc_next = mk_sc(0)
                        for ki in range(KT):
                            sc = sc_next
                            if ki + 1 < KT:
                                pump(base + ki + 1,
                                     32 - qc * 16 - ki - 1)
                                sc_next = mk_sc(ki + 1)
                            et = etp.tile([P, 1024], BF, tag="et")
                            nc.scalar.activation(
                                et[:], sc[:], AF.Exp,
                                scale=float(1 / np.sqrt(HD)))
                            if ki == 2 and pending_norm:
                                pending_norm.popleft()()
                            nc.tensor.matmul(
                                ps_e[0:65, :], v_sb[:, ki, pl, 0:65],
                                et[:, 0:512],
                                start=(ki == 0), stop=(ki == KT - 1))
                            nc.tensor.matmul(
                                ps_d[0:65, :], v_sb[:, ki, pl, 65:130],
                                et[:, 512:1024],
                                start=(ki == 0), stop=(ki == KT - 1))
                        # evict unnormalized O (+den row) on ACT engine
                        oe = gbp.tile([65, 512], F32, tag="oe")
                        nc.scalar.activation(oe[:], ps_e[0:65, :], AF.Copy)
                        od = gbp.tile([65, 512], F32, tag="od")
                        nc.scalar.activation(od[:], ps_d[0:65, :], AF.Copy)
                        o_t[(p, qc)] = (oe, od)
                        if p == NPAIR - 1 and qc == 1:
                            while pending_norm:
                                pending_norm.popleft()()
                            emit_norm(p, qc)
                        else:
                            pending_norm.append(
                                lambda p=p, qc=qc: emit_norm(p, qc))
                while pending_norm:
                    pending_norm.popleft()()
                while todo:
                    todo.popleft()[1]()

            if phases == "att":
                nc.sync.dma_start(outT[0:P, 0:512],
                                  osb_big.bitcast(F32)[:, 0, 0:512])
                return

            # ======== P2: out-proj + residual + LN1 ========
            # Residual written in place over x_sb's own-token columns;
            # "s" below is x_sb[:, d, 0:1024].
            def ln1_finish_d(qc, d, mean_b, rstd_b):
                qs = slice(qc * 512, (qc + 1) * 512)
                nc.vector.tensor_sub(x_sb[:, d, qs], x_sb[:, d, qs],
                                     mean_b[:])
                if f["ln1"]:
                    tmp = finp.tile([P, 512], F32, tag="ftmp")
                    nc.vector.tensor_mul(tmp[:], x_sb[:, d, qs],
                                         rstd_b[:])
                    nc.vector.tensor_scalar(
                        x_sb[:, d, qs], tmp[:], g1_sb[:, d:d + 1],
                        be1_sb[:, d:d + 1], ALU.mult, ALU.add)
                else:
                    nc.vector.tensor_mul(x_sb[:, d, qs],
                                         x_sb[:, d, qs], rstd_b[:])

            chain1 = {}
            p2 = contextlib.ExitStack()
            with p2:
                wop = p2.enter_context(tc.tile_pool(name="wop", bufs=8))
                sqp = p2.enter_context(tc.tile_pool(name="sqp", bufs=2))
                ps_ac = p2.enter_context(
                    tc.tile_pool(name="ps_ac", bufs=3, space="PSUM"))
                ps_st = p2.enter_context(
                    tc.tile_pool(name="ps_st", bufs=2, space="PSUM"))

                wo_t = []
                for d in range(DT):
                    wt = wop.tile([P, DT, P], BF, tag="wo")
                    dma_engs[d % 3].dma_start(wt[:], Wo[d])
                    wo_t.append(wt)

                sq_t = {}
                for qc in range(2):
                    qs = slice(qc * 512, (qc + 1) * 512)
                    st_s = ps_st.tile([1, 512], F32, tag="st")
                    st_q = ps_st.tile([1, 512], F32, tag="st")

                    def st_mm(d, qs=qs, st_s=st_s, st_q=st_q):
                        nc.tensor.matmul(st_s[:], ones_cb[:],
                                         x_sb[:, d, qs],
                                         start=(d == 0),
                                         stop=(d == DT - 1))
                        nc.tensor.matmul(st_q[:], ones_cf[:], sq_t[d][:],
                                         start=(d == 0),
                                         stop=(d == DT - 1))

                    for d in range(DT):
                        ps = ps_ac.tile([P, 512], F32, tag="ac")
                        for pr in range(NPAIR):
                            nc.tensor.matmul(ps[:], wo_t[d][:, pr, :],
                                             osb_big[:, pr, qs],
                                             start=(pr == 0),
                                             stop=(pr == NPAIR - 1))
                        if f["bo"]:
                            tmp = finp.tile([P, 512], F32, tag="ftmp")
                            nc.vector.tensor_scalar_add(
                                tmp[:], ps[:], bo_sb[:, d:d + 1])
                            nc.vector.tensor_add(
                                x_sb[:, d, qs], tmp[:], x_sb[:, d, qs])
                        else:
                            nc.vector.tensor_add(x_sb[:, d, qs], ps[:],
                                                 x_sb[:, d, qs])
                        sq = sqp.tile([P, 512], FR, tag="sq")
                        nc.scalar.square(sq[:], x_sb[:, d, qs])
                        sq_t[d] = sq
                        if d >= 1:
                            st_mm(d - 1)
                        if qc == 1 and d >= 1:
                            ln1_finish_d(0, d - 1, *chain1[0])
                    st_mm(DT - 1)
                    chain1[qc] = ln_chain(st_s, st_q)
                    if qc == 1:
                        ln1_finish_d(0, DT - 1, *chain1[0])

            if phases == "p2":
                nc.sync.dma_start(outT[0:P, 0:512],
                                  x_sb.bitcast(F32)[:, 0, 0:512])
                return

            # ======== P3: FFN (x_sb[:, :, 0:1024] holds h) ========
            p3 = contextlib.ExitStack()
            with p3:
                w1p = p3.enter_context(tc.tile_pool(name="w1p", bufs=8))
                ffp = p3.enter_context(tc.tile_pool(name="ffp", bufs=2))
                sqp3 = p3.enter_context(tc.tile_pool(name="sqp3", bufs=2))
                w2p = p3.enter_context(tc.tile_pool(name="w2p", bufs=4))
                ps_ac3 = p3.enter_context(
                    tc.tile_pool(name="ps_ac3", bufs=3, space="PSUM"))
                ps_st3 = p3.enter_context(
                    tc.tile_pool(name="ps_st3", bufs=2, space="PSUM"))
                ps_bc = p3.enter_context(
                    tc.tile_pool(name="ps_bc", bufs=2, space="PSUM"))
                NQ = 4
                FQ = FT // NQ
                # s2 reuses osb_big's pool slot (osb dead after out-proj)
                s2 = obp.tile([P, DT, T_MY], FR, tag="ob", name="s2")

                def ffn1_step(fo, fo_l, qc, w1t):
                    qs = slice(qc * 512, (qc + 1) * 512)
                    ps = ps_ac3.tile([P, 512], F32, tag="ac")
                    for k in range(DT):
                        nc.tensor.matmul(ps[:], w1t[:, k, :],
                                         x_sb[:, k, qs],
                                         start=(k == 0),
                                         stop=(k == DT - 1))
                    nc.scalar.activation(
                        ff1q[:, fo_l, qs], ps[:], AF.Relu,
                        bias=(b1_sb[:, fo:fo + 1] if f["b1"] else 0.0))

                def ffn2_step(quarter, d, qc, w2t):
                    qs = slice(qc * 512, (qc + 1) * 512)
                    ps = ps_ac3.tile([P, 512], F32, tag="ac")
                    for k in range(FQ):
                        nc.tensor.matmul(ps[:], w2t[:, k, :],
                                         ff1q[:, k, qs],
                                         start=(k == 0),
                                         stop=(k == FQ - 1))
                    if quarter == 0:
                        if f["b2"]:
                            nc.vector.tensor_scalar_add(
                                s2[:, d, qs], ps[:], b2_sb[:, d:d + 1])
                            nc.vector.tensor_add(
                                s2[:, d, qs], s2[:, d, qs],
                                x_sb[:, d, qs])
                        else:
                            nc.vector.tensor_add(s2[:, d, qs], ps[:],
                                                 x_sb[:, d, qs])
                    else:
                        nc.vector.tensor_add(s2[:, d, qs],
                                             s2[:, d, qs], ps[:])

                def ln2_finish_d(qc, d, mean_b, rstd_b, eng=None):
                    eng = eng or nc.vector
                    qs = slice(qc * 512, (qc + 1) * 512)
                    eng.tensor_sub(s2[:, d, qs], s2[:, d, qs],
                                   mean_b[:])
                    if f["ln2"]:
                        tmp = finp.tile([P, 512], F32, tag="ftmp")
                        eng.tensor_mul(tmp[:], s2[:, d, qs],
                                       rstd_b[:])
                        eng.tensor_scalar(
                            s2[:, d, qs], tmp[:], g2_sb[:, d:d + 1],
                            be2_sb[:, d:d + 1], ALU.mult, ALU.add)
                    else:
                        eng.tensor_mul(s2[:, d, qs],
                                       s2[:, d, qs], rstd_b[:])
                    nc.sync.dma_start(outT[d * P:(d + 1) * P, qs],
                                      s2[:, d, qs].bitcast(F32))

                for quarter in range(NQ):
                    ff1q = ffp.tile([P, FQ, T_MY], BF, tag="ff1")
                    if quarter == 0:
                        # qc-major: qc0 matmuls overlap LN1-qc1 finish
                        w1ts = []
                        for fo_l in range(FQ):
                            w1t = w1p.tile([P, DT, P], BF, tag="w1")
                            dma_engs[fo_l % 3].dma_start(
                                w1t[:], W1[quarter * FQ + fo_l])
                            w1ts.append(w1t)
                        for qc in range(2):
                            for fo_l in range(FQ):
                                ffn1_step(quarter * FQ + fo_l, fo_l, qc,
                                          w1ts[fo_l])
                                if qc == 0:
                                    ln1_finish_d(1, fo_l, *chain1[1])
                    else:
                        for fo_l in range(FQ):
                            fo = quarter * FQ + fo_l
                            w1t = w1p.tile([P, DT, P], BF, tag="w1")
                            dma_engs[fo_l % 3].dma_start(w1t[:], W1[fo])
                            for qc in range(2):
                                ffn1_step(fo, fo_l, qc, w1t)
                    if quarter < NQ - 1:
                        for d in range(DT):
                            w2t = w2p.tile([P, FQ, P], BF, tag="w2")
                            dma_engs[d % 3].dma_start(w2t[:],
                                                      W2[quarter, d])
                            for qc in range(2):
                                ffn2_step(quarter, d, qc, w2t)
                    else:
                        chain2 = {}
                        for qc in range(2):
                            qs = slice(qc * 512, (qc + 1) * 512)
                            st_s = ps_st3.tile([1, 512], F32, tag="st")
                            st_q = ps_st3.tile([1, 512], F32, tag="st")
                            sq3_t = {}

                            def st3_mm(d, qs=qs, st_s=st_s, st_q=st_q,
                                       sq3_t=sq3_t):
                                nc.tensor.matmul(st_s[:], ones_cf[:],
                                                 s2[:, d, qs],
                                                 start=(d == 0),
                                                 stop=(d == DT - 1))
                                nc.tensor.matmul(st_q[:], ones_cf[:],
                                                 sq3_t[d][:],
                                                 start=(d == 0),
                                                 stop=(d == DT - 1))

                            for d in range(DT):
                                w2t = w2p.tile([P, FQ, P], BF, tag="w2")
                                dma_engs[d % 3].dma_start(
                                    w2t[:], W2[quarter, d])
                                ffn2_step(quarter, d, qc, w2t)
                                sq = sqp3.tile([P, 512], FR, tag="sq3")
                                nc.scalar.square(sq[:], s2[:, d, qs])
                                sq3_t[d] = sq
                                if d >= 1:
                                    st3_mm(d - 1)
                                if qc == 1 and d >= 1:
                                    ln2_finish_d(0, d - 1, *chain2[0])
                            st3_mm(DT - 1)
                            chain2[qc] = ln_chain(
                                st_s, st_q,
                                ps_pool=(ps_bc if qc == 1 else None))
                            if qc == 1:
                                ln2_finish_d(0, DT - 1, *chain2[0])
                        for d in range(DT):
                            ln2_finish_d(1, d, *chain2[1])


# ---------------- host-side helpers ----------------

def shard_inputs(inputs):
    import ml_dtypes
    bf16 = ml_dtypes.bfloat16
    x = np.asarray(inputs["x"], dtype=np.float32)
    shared = {k: np.ascontiguousarray(np.asarray(inputs[k], np.float32))
              for k in ("bq", "bk", "bo", "b1", "b2", "g1", "be1",
                        "g2", "be2")}
    shared["bv"] = np.ascontiguousarray(
        np.asarray(inputs["bv"], np.float32)).astype(bf16)
    Wq = np.asarray(inputs["Wq"], np.float32)
    Wk = np.asarray(inputs["Wk"], np.float32)
    Wv = np.asarray(inputs["Wv"], np.float32)
    Wo = np.asarray(inputs["Wo"], np.float32)
    W1 = np.asarray(inputs["W1"], np.float32)
    W2 = np.asarray(inputs["W2"], np.float32)
    FQ = FT // 4
    shared["Wq"] = np.ascontiguousarray(
        Wq.reshape(DT, P, NPAIR, P).transpose(2, 1, 0, 3)).astype(bf16)
    shared["Wk"] = np.ascontiguousarray(
        Wk.reshape(DT, P, NPAIR, P).transpose(2, 1, 0, 3)).astype(bf16)
    shared["Wv"] = np.ascontiguousarray(
        Wv.reshape(DT, P, 4, 256).transpose(2, 1, 0, 3)).astype(bf16)
    shared["Wo"] = np.ascontiguousarray(
        Wo.reshape(NPAIR, P, DT, P).transpose(2, 1, 0, 3)).astype(bf16)
    shared["W1"] = np.ascontiguousarray(
        W1.reshape(DT, P, FT, P).transpose(2, 1, 0, 3)).astype(bf16)
    shared["W2"] = np.ascontiguousarray(
        W2.reshape(4, FQ, P, DT, P).transpose(0, 3, 2, 1, 4)).astype(bf16)
    maps = []
    for c in range(N_CORES):
        b, h = c // 2, c % 2
        xTb = x[b].T
        roll = np.concatenate([xTb[:, h * T_MY:], xTb[:, :h * T_MY]], axis=1)
        m = {"xT": np.ascontiguousarray(roll).astype(bf16)}
        m.update(shared)
        maps.append(m)
    return maps


def unshard_output(results):
    out = np.empty((B, S, D), np.float32)
    for c in range(N_CORES):
        b, h = c // 2, c % 2
        out[b, h * T_MY:(h + 1) * T_MY, :] = results[c]["outT"].T
    return out


def flags_from_inputs(inputs):
    def nz(k):
        return bool(np.any(np.asarray(inputs[k])))

    return {
        "bq": nz("bq"), "bk": nz("bk"), "bv": nz("bv"), "bo": nz("bo"),
        "b1": nz("b1"), "b2": nz("b2"),
        "ln1": nz("be1") or not np.allclose(np.asarray(inputs["g1"]), 1.0),
        "ln2": nz("be2") or not np.allclose(np.asarray(inputs["g2"]), 1.0),
    }


# ---------------- SPMD runner ----------------


import time

import jax
from jax.sharding import Mesh, PartitionSpec
from jax.experimental.shard_map import shard_map

import concourse.bass2jax as b2j


class SpmdRunner:
    def __init__(self, nc, n_cores: int):
        b2j.install_neuronx_cc_hook()
        self.nc = nc
        self.n_cores = n_cores

        partition_name = (
            nc.partition_id_tensor.name if nc.partition_id_tensor else None
        )
        in_names, out_names, out_avals, zero_outs = [], [], [], []
        for alloc in nc.m.functions[0].allocations:
            if not isinstance(alloc, mybir.MemoryLocationSet):
                continue
            name = alloc.memorylocations[0].name
            if alloc.kind == "ExternalInput":
                if name != partition_name:
                    in_names.append(name)
            elif alloc.kind == "ExternalOutput":
                shape = tuple(alloc.tensor_shape)
                dtype = mybir.dt.np(alloc.dtype)
                out_names.append(name)
                out_avals.append(jax.core.ShapedArray(shape, dtype))
                zero_outs.append(np.zeros(shape, dtype))
        self.in_names, self.out_names = in_names, out_names
        self.out_avals = out_avals
        n_params, n_outs = len(in_names), len(out_names)
        self.n_params = n_params

        all_in_names = list(in_names) + list(out_names)
        if partition_name is not None:
            all_in_names.append(partition_name)

        def _body(*args):
            operands = list(args)
            if partition_name is not None:
                operands.append(b2j.partition_id_tensor())
            outs = b2j._bass_exec_p.bind(
                *operands,
                out_avals=tuple(out_avals),
                in_names=tuple(all_in_names),
                out_names=tuple(out_names),
                lowering_input_output_aliases=(),
                sim_require_finite=True,
                sim_require_nnan=True,
                nc=nc,
            )
            return tuple(outs)

        devices = jax.devices()[:n_cores]
        self.mesh = Mesh(np.asarray(devices), ("core",))
        in_specs = (PartitionSpec("core"),) * (n_params + n_outs)
        out_specs = (PartitionSpec("core"),) * n_outs
        # No donation: keeps zero-out buffers reusable across repeated calls.
        self.fn = jax.jit(
            shard_map(
                _body,
                mesh=self.mesh,
                in_specs=in_specs,
                out_specs=out_specs,
                check_rep=False,
            ),
            keep_unused=True,
        )
        self.zero_outs = zero_outs
        self._dev_zeros = None

    def put_inputs(self, in_maps: list[dict[str, np.ndarray]]):
        """Concat per-core inputs on axis 0 and move to device once."""
        concat = [
            np.concatenate(
                [np.asarray(in_maps[c][n]) for c in range(self.n_cores)], axis=0
            )
            for n in self.in_names
        ]
        sharding = jax.sharding.NamedSharding(self.mesh, PartitionSpec("core"))
        dev_in = [jax.device_put(a, sharding) for a in concat]
        if self._dev_zeros is None:
            self._dev_zeros = [
                jax.device_put(
                    np.zeros((self.n_cores * z.shape[0], *z.shape[1:]), z.dtype),
                    sharding,
                )
                for z in self.zero_outs
            ]
        return dev_in

    def run(self, dev_in):
        outs = self.fn(*dev_in, *self._dev_zeros)
        jax.block_until_ready(outs)
        return outs

    def run_numpy(self, in_maps):
        dev_in = self.put_inputs(in_maps)
        outs = self.run(dev_in)
        res = []
        for c in range(self.n_cores):
            d = {}
            for i, name in enumerate(self.out_names):
                full = np.asarray(outs[i])
                per = full.reshape(self.n_cores, *self.out_avals[i].shape)
                d[name] = per[c]
            res.append(d)
        return res

    def time_runs(self, dev_in, n=10, warmup=2):
        for _ in range(warmup):
            self.run(dev_in)
        times = []
        for _ in range(n):
            t0 = time.perf_counter()
            self.run(dev_in)
            times.append(time.perf_counter() - t0)
        return times


# ---------------- public entry point ----------------

_CACHE = {}


def _get_runner(flag_key, flags):
    if flag_key not in _CACHE:
        nc = build_encoder(flags)
        _CACHE[flag_key] = SpmdRunner(nc, N_CORES)
    return _CACHE[flag_key]


def kernel(**inputs):
    """Full-input encoder layer on 8 NeuronCores; returns [B, S, D] f32."""
    flags = flags_from_inputs(inputs)
    key = tuple(sorted(flags.items()))
    in_maps = shard_inputs(inputs)
    try:
        runner = _get_runner(key, flags)
        results = runner.run_numpy(in_maps)
    except Exception:
        # Device/mesh hiccup: reset backends and retry once from scratch.
        _CACHE.clear()
        try:
            jax.clear_caches()
        except Exception:
            pass
        try:
            jax.extend.backend.clear_backends()
        except Exception:
            pass
        runner = _get_runner(key, flags)
        results = runner.run_numpy(in_maps)
    return unshard_output(results)


# revision 17
# speedup vs baseline: 1.1147x; 1.0010x over previous
"""Encoder-layer Bass/Tile kernel for TRN2, data-parallel over 8 cores.

Layout strategy: feature-major ("transposed") activations throughout.
Core c handles batch b = c//2, sequence half h = c%2 (1024 query tokens).
Host rotates each core's x^T so that *its* tokens are always columns
0:1024 — the program is identical across cores (pure SPMD); attention is
permutation-invariant over keys so the rotated K/V order is harmless.

x and all weights are bf16 (full PE rate, halves SBUF + DMA); PSUM
accumulation, LayerNorm statistics and the final residual stream s2 are
fp32.  Measured end-to-end error vs the fp32 reference ~3e-3.

Per-core pipeline:
  P0/P1 attention: x streamed block-major so V/K/Q projection matmuls
    start as soon as block 0 lands.  Per head-pair: scores^T = K_h Q_h^T
    (two heads in PE row-groups), exp on ACT (scale=1/8, no
    max-subtraction), O^T accumulated over key tiles with a ones column
    per head (softmax row-sum trick).  The next pair's K/Q/V projection
    matmuls are *interleaved into the ki loop* (thunk pump) so the
    tensor queue never drains while ACT computes exp.  PSUM eviction on
    ACT (Copy); softmax normalization via reciprocal_approx_fast +
    gpsimd broadcast, deferred one qc-slot.
  P2 out-proj + residual + LN1, residual written in place over x^T's
    own-token columns; LN stats matmuls interleaved one d-tile behind
    the projection stream; LN1-qc0 finish overlaps qc1's matmuls,
    LN1-qc1 finish overlaps the first FFN1 quarter.
  P3 FFN in d_ff quarters (ReLU on ACT eviction); LN2 stats interleaved
    into the last quarter; qc0's normalize+store overlaps qc1's FFN2.
"""

import contextlib
from collections import deque

import numpy as np

import concourse.bass as bass  # noqa: F401
import concourse.mybir as mybir
import concourse.tile as tile
from concourse import bacc

N_CORES = 8
P = 128
D = 1024
DFF = 4096
H = 16
HD = 64
NPAIR = 8
T_MY = 1024
T_KV = 2048
DT = D // P
FT = DFF // P
KT = T_KV // P
EPS = 1e-5
B, S = 4, 2048

F32 = mybir.dt.float32
FR = mybir.dt.float32r
BF = mybir.dt.bfloat16
AF = mybir.ActivationFunctionType
ALU = mybir.AluOpType

DEFAULT_FLAGS = {
    "bq": False, "bk": False, "bv": False, "bo": False,
    "b1": False, "b2": False, "ln1": False, "ln2": False,
}


def build_encoder(flags=None, hw_reps=1, phases='all'):
    f = dict(DEFAULT_FLAGS)
    if flags:
        f.update(flags)

    nc = bacc.Bacc("TRN2", target_bir_lowering=False, debug=False,
                   num_devices=N_CORES)

    xT = nc.dram_tensor("xT", [D, T_KV], BF, kind="ExternalInput")
    # weights arrive host-pre-tiled (bf16) so every DMA is one contiguous
    # run per partition
    Wq = nc.dram_tensor("Wq", [NPAIR, P, DT, P], BF, kind="ExternalInput")
    Wk = nc.dram_tensor("Wk", [NPAIR, P, DT, P], BF, kind="ExternalInput")
    Wv = nc.dram_tensor("Wv", [4, P, DT, 256], BF, kind="ExternalInput")
    Wo = nc.dram_tensor("Wo", [DT, P, NPAIR, P], BF, kind="ExternalInput")
    W1 = nc.dram_tensor("W1", [FT, P, DT, P], BF, kind="ExternalInput")
    W2 = nc.dram_tensor("W2", [4, DT, P, FT // 4, P], BF,
                        kind="ExternalInput")
    bq = nc.dram_tensor("bq", [D], F32, kind="ExternalInput")
    bk = nc.dram_tensor("bk", [D], F32, kind="ExternalInput")
    bv = nc.dram_tensor("bv", [D], BF, kind="ExternalInput")
    bo = nc.dram_tensor("bo", [D], F32, kind="ExternalInput")
    b1 = nc.dram_tensor("b1", [DFF], F32, kind="ExternalInput")
    b2 = nc.dram_tensor("b2", [D], F32, kind="ExternalInput")
    g1 = nc.dram_tensor("g1", [D], F32, kind="ExternalInput")
    be1 = nc.dram_tensor("be1", [D], F32, kind="ExternalInput")
    g2 = nc.dram_tensor("g2", [D], F32, kind="ExternalInput")
    be2 = nc.dram_tensor("be2", [D], F32, kind="ExternalInput")
    outT = nc.dram_tensor("outT", [D, T_MY], F32, kind="ExternalOutput")

    tensors = dict(
        xT=xT, Wq=Wq, Wk=Wk, Wv=Wv, Wo=Wo, W1=W1, W2=W2, bq=bq, bk=bk,
        bv=bv, bo=bo, b1=b1, b2=b2, g1=g1, be1=be1, g2=g2, be2=be2,
        outT=outT)

    with tile.TileContext(nc) as tc:
        if hw_reps > 1:
            with tc.For_i(0, hw_reps, 1):
                _body(nc, tc, tensors, f, phases)
        else:
            _body(nc, tc, tensors, f, phases)
    nc.compile()
    return nc


def _body(nc, tc, t, f, phases='all'):
    xT, Wq, Wk, Wv, Wo, W1, W2 = (t[k] for k in
                                  ("xT", "Wq", "Wk", "Wv", "Wo", "W1", "W2"))
    bq, bk, bv, bo, b1, b2 = (t[k] for k in ("bq", "bk", "bv", "bo", "b1",
                                             "b2"))
    g1, be1, g2, be2 = (t[k] for k in ("g1", "be1", "g2", "be2"))
    outT = t["outT"]

    ctx = contextlib.ExitStack()
    with ctx:
        ctx.enter_context(nc.allow_low_precision(
            reason="bf16 weights/activations are intended; stats stay f32"))
        const = ctx.enter_context(tc.tile_pool(name="const", bufs=1))
        onesF = const.tile([P, P], F32)
        nc.vector.memset(onesF[:], 1.0)
        # stats stationaries pre-scaled by 1/D so the matmul yields the
        # mean / mean-square directly (1/1024 is exact in bf16)
        ones_cb = const.tile([P, 1], BF)
        nc.vector.tensor_scalar_mul(ones_cb[:], onesF[:, 0:1], 1.0 / D)
        ones_cf = const.tile([P, 1], FR)
        nc.vector.tensor_scalar_mul(ones_cf[:], onesF[:, 0:1], 1.0 / D)
        ones_rb = const.tile([1, P], BF)
        nc.vector.tensor_copy(ones_rb[:], onesF[0:1, :])
        eps_c = const.tile([1, 1], F32)
        nc.vector.memset(eps_c[:], EPS)

        def vec_tile(pool, name, src, n):
            tl = pool.tile([P, n], F32, name=name)
            nc.vector.dma_start(tl[:], src.rearrange("(t p) -> p t", p=P))
            return tl

        bq_sb = vec_tile(const, "bq_sb", bq, DT) if f["bq"] else None
        bk_sb = vec_tile(const, "bk_sb", bk, DT) if f["bk"] else None
        bo_sb = vec_tile(const, "bo_sb", bo, DT) if f["bo"] else None
        b1_sb = vec_tile(const, "b1_sb", b1, FT) if f["b1"] else None
        b2_sb = vec_tile(const, "b2_sb", b2, DT) if f["b2"] else None
        g1_sb = vec_tile(const, "g1_sb", g1, DT) if f["ln1"] else None
        be1_sb = vec_tile(const, "be1_sb", be1, DT) if f["ln1"] else None
        g2_sb = vec_tile(const, "g2_sb", g2, DT) if f["ln2"] else None
        be2_sb = vec_tile(const, "be2_sb", be2, DT) if f["ln2"] else None
        if f["bv"]:
            bv_sb = const.tile([1, D], BF)
            nc.vector.dma_start(bv_sb[:], bv[None, :])

        # small pools whose tiles span P2 -> P3
        finp = ctx.enter_context(tc.tile_pool(name="finp", bufs=2))
        bbp = ctx.enter_context(tc.tile_pool(name="bbp", bufs=2))
        rwp = ctx.enter_context(tc.tile_pool(name="rwp", bufs=1))
        rws = ctx.enter_context(tc.tile_pool(name="rws", bufs=1))

        dma_engs = [nc.sync, nc.scalar, nc.gpsimd]

        def recip(out_ap, in_ap):
            # reciprocal_approx_fast reads partition 0 only — callers must
            # pass base-partition-0 sources (~18 correct bits, 5x faster)
            nc.vector.reciprocal_approx_fast(out=out_ap, in_=in_ap)

        def ln_chain(st_s, st_q, ps_pool=None):
            """PSUM stats (already /D) -> broadcast mean/rstd [P,512].

            ps_pool set: broadcast via K=1 matmuls on the (idle) tensor
            engine into PSUM instead of gpsimd — used for the tail chain.
            """
            mean = rwp.tile([1, 512], F32, tag="mean")
            nc.vector.tensor_copy(mean[:], st_s[:])
            m2 = rws.tile([1, 512], F32, tag="m2")
            nc.vector.tensor_mul(m2[:], mean[:], mean[:])
            var = rws.tile([1, 512], F32, tag="var")
            nc.vector.tensor_sub(var[:], st_q[:], m2[:])
            sd = rws.tile([1, 512], F32, tag="sd")
            nc.scalar.activation(sd[:], var[:], AF.Sqrt, bias=eps_c[:])
            rstd = rwp.tile([1, 512], F32, tag="rstd")
            recip(rstd[:], sd[:])
            if ps_pool is not None:
                mean_b = ps_pool.tile([P, 512], F32, tag="bc")
                nc.tensor.matmul(mean_b[:], onesF[0:1, :], mean[:],
                                 start=True, stop=True)
                rstd_b = ps_pool.tile([P, 512], F32, tag="bc")
                nc.tensor.matmul(rstd_b[:], onesF[0:1, :], rstd[:],
                                 start=True, stop=True)
            else:
                mean_b = bbp.tile([P, 512], F32, tag="mb")
                nc.gpsimd.partition_broadcast(mean_b[:], mean[:])
                rstd_b = bbp.tile([P, 512], F32, tag="rb")
                nc.gpsimd.partition_broadcast(rstd_b[:], rstd[:])
            return mean_b, rstd_b

        m2s = contextlib.ExitStack()
        with m2s:
            xp = m2s.enter_context(tc.tile_pool(name="xp", bufs=1))
            x_sb = xp.tile([P, DT, T_KV], BF)
            obp = m2s.enter_context(tc.tile_pool(name="obp", bufs=1))
            osb_big = obp.tile([P, NPAIR, T_MY], BF, tag="ob",
                               name="osb_big")
            xTv = xT.rearrange("(kt p) t -> p kt t", p=P)

            att = contextlib.ExitStack()
            with att:
                vp = att.enter_context(tc.tile_pool(name="vp", bufs=2))
                wvp = att.enter_context(tc.tile_pool(name="wvp", bufs=2))
                wqkp = att.enter_context(tc.tile_pool(name="wqkp", bufs=2))
                qkp = att.enter_context(tc.tile_pool(name="qkp", bufs=2))
                etp = att.enter_context(tc.tile_pool(name="etp", bufs=2))
                gbp = att.enter_context(tc.tile_pool(name="gbp", bufs=2))
                nrm = att.enter_context(tc.tile_pool(name="nrm", bufs=2))
                ps_sc = att.enter_context(
                    tc.tile_pool(name="ps_sc", bufs=2, space="PSUM"))
                ps_o = att.enter_context(
                    tc.tile_pool(name="ps_o", bufs=2, space="PSUM"))
                ps_pr = att.enter_context(
                    tc.tile_pool(name="ps_pr", bufs=2, space="PSUM"))

                wv_t, v_t, kq_t = {}, {}, {}

                def v_dma(grp):
                    wv_sb = wvp.tile([P, DT, 256], BF, tag="wv",
                                     name="wv_sb")
                    nc.sync.dma_start(wv_sb[:, 0:DT // 2, :],
                                      Wv[grp][:, 0:DT // 2, :])
                    nc.scalar.dma_start(wv_sb[:, DT // 2:DT, :],
                                        Wv[grp][:, DT // 2:DT, :])
                    wv_t[grp] = wv_sb
                    v_t[grp] = vp.tile([P, KT, 2, 130], BF, tag="v",
                                       name="v_sb")

                def kq_dma(p):
                    wq_p = wqkp.tile([P, DT, P], BF, tag="wq", name="wq_p")
                    nc.gpsimd.dma_start(wq_p[:], Wq[p])
                    wk_p = wqkp.tile([P, DT, P], BF, tag="wk", name="wk_p")
                    nc.gpsimd.dma_start(wk_p[:], Wk[p])
                    kt_sb = qkp.tile([P, 4, 512], BF, tag="kt",
                                     name="kt_sb")
                    qt_sb = qkp.tile([P, 2, 512], BF, tag="qt",
                                     name="qt_sb")
                    kq_t[p] = (wq_p, wk_p, kt_sb, qt_sb)

                def v_ones(grp):
                    v_sb = v_t[grp]
                    ones_src = onesF[:, 0:32].rearrange(
                        "p (a b o) -> p a b o", a=KT, b=2)
                    nc.vector.tensor_copy(v_sb[:, :, :, 64:65], ones_src)
                    nc.vector.tensor_copy(v_sb[:, :, :, 129:130], ones_src)

                def v_step(grp, tt):
                    wv_sb, v_sb = wv_t[grp], v_t[grp]
                    ps = ps_pr.tile([P, 256], F32, tag="pr", name="psv")
                    for k in range(DT):
                        nc.tensor.matmul(
                            ps[:], x_sb[:, k, tt * P:(tt + 1) * P],
                            wv_sb[:, k, :],
                            start=(k == 0),
                            stop=(k == DT - 1 and not f["bv"]))
                    if f["bv"]:
                        nc.tensor.matmul(
                            ps[:], ones_rb[:],
                            bv_sb[:, grp * 256:(grp + 1) * 256],
                            start=False, stop=True)
                    dst = v_sb[:, tt, :, :].rearrange(
                        "p pr (hip j) -> p pr hip j", hip=2)[:, :, :, 0:64]
                    src = ps.rearrange("p (pr hip j) -> p pr hip j",
                                       pr=2, hip=2)
                    nc.vector.tensor_copy(dst, src)

                def kt_step(p, c):
                    _, wk_p, kt_sb, _ = kq_t[p]
                    ps = ps_pr.tile([P, 512], F32, tag="pr", name="psk")
                    for k in range(DT):
                        nc.tensor.matmul(
                            ps[:], wk_p[:, k, :],
                            x_sb[:, k, c * 512:(c + 1) * 512],
                            start=(k == 0), stop=(k == DT - 1))
                    if f["bk"]:
                        nc.vector.tensor_scalar_add(
                            kt_sb[:, c, :], ps[:], bk_sb[:, p:p + 1])
                    else:
                        nc.vector.tensor_copy(kt_sb[:, c, :], ps[:])

                def qt_step(p, c):
                    wq_p, _, _, qt_sb = kq_t[p]
                    ps = ps_pr.tile([P, 512], F32, tag="pr", name="psq")
                    for k in range(DT):
                        nc.tensor.matmul(
                            ps[:], wq_p[:, k, :],
                            x_sb[:, k, c * 512:(c + 1) * 512],
                            start=(k == 0), stop=(k == DT - 1))
                    if f["bq"]:
                        nc.vector.tensor_scalar_add(
                            qt_sb[:, c, :], ps[:], bq_sb[:, p:p + 1])
                    else:
                        nc.vector.tensor_copy(qt_sb[:, c, :], ps[:])

                def kq_thunks(p):
                    return ([lambda c=c: kt_step(p, c) for c in range(4)]
                            + [lambda c=c: qt_step(p, c) for c in range(2)])

                def v_thunks(grp):
                    out = [lambda: (v_ones(grp), v_step(grp, 0))]
                    out += [lambda tt=tt: v_step(grp, tt)
                            for tt in range(1, KT)]
                    return out

                # Wv + x block 0 first (V projection starts the kernel),
                # then pair-0 K/Q weights, then the remaining x blocks
                v_dma(0)
                for k in range(DT):
                    dma_engs[k % 3].dma_start(
                        x_sb[:, k, 0:512], xTv[:, k, 0:512])
                kq_dma(0)
                for blk in range(1, 4):
                    for k in range(DT):
                        dma_engs[(blk * DT + k) % 3].dma_start(
                            x_sb[:, k, blk * 512:(blk + 1) * 512],
                            xTv[:, k, blk * 512:(blk + 1) * 512])

                # prologue: just enough to start pair-0 qc0
                v_ones(0)
                for tt in range(4):
                    v_step(0, tt)
                kt_step(0, 0)
                qt_step(0, 0)
                qt_step(0, 1)

                todo = deque()
                todo.extend([
                    (3, lambda: kt_step(0, 1)),
                    (4, lambda: v_step(0, 4)),
                    (5, lambda: v_step(0, 5)),
                    (6, lambda: v_step(0, 6)),
                    (7, lambda: kt_step(0, 2)),
                    (7, lambda: v_step(0, 7)),
                    (8, lambda: v_step(0, 8)),
                    (9, lambda: v_step(0, 9)),
                    (10, lambda: v_step(0, 10)),
                    (11, lambda: kt_step(0, 3)),
                    (11, lambda: v_step(0, 11)),
                    (12, lambda: v_step(0, 12)),
                    (13, lambda: v_step(0, 13)),
                    (14, lambda: v_step(0, 14)),
                    (15, lambda: v_step(0, 15)),
                ])

                pending_norm = deque()
                o_t = {}

                def emit_norm(p, qc):
                    oe, od = o_t.pop((p, qc))
                    qs = slice(qc * 512, (qc + 1) * 512)
                    den = nrm.tile([1, 1024], F32, tag="den")
                    nc.vector.tensor_copy(den[0:1, 0:512], oe[64:65, :])
                    nc.vector.tensor_copy(den[0:1, 512:1024], od[64:65, :])
                    rr = nrm.tile([1, 1024], F32, tag="rr")
                    recip(rr[:], den[:])
                    bc_e = nrm.tile([HD, 512], F32, tag="bce")
                    nc.gpsimd.partition_broadcast(bc_e[:], rr[0:1, 0:512])
                    bc_d = nrm.tile([HD, 512], F32, tag="bce")
                    nc.gpsimd.partition_broadcast(bc_d[:], rr[0:1, 512:1024])
                    nc.vector.tensor_mul(osb_big[0:HD, p, qs],
                                         oe[0:HD, :], bc_e[:])
                    nc.vector.tensor_mul(osb_big[HD:P, p, qs],
                                         od[0:HD, :], bc_d[:])

                for p in range(NPAIR):
                    grp, pl = p // 2, p % 2
                    if p + 1 < NPAIR:
                        kq_dma(p + 1)
                        if p + 1 == NPAIR - 1:
                            # spread pair-7's projections into the
                            # otherwise-starved pair-6/7 exp bubbles
                            todo.extend([
                                (208, lambda: kt_step(7, 0)),
                                (214, lambda: qt_step(7, 0)),
                                (227, lambda: kt_step(7, 1)),
                                (231, lambda: kt_step(7, 2)),
                                (235, lambda: kt_step(7, 3)),
                                (238, lambda: qt_step(7, 1)),
                            ])
                        else:
                            todo.extend((-1, fn)
                                        for fn in kq_thunks(p + 1))
                        if (p + 1) % 2 == 0:
                            g = (p + 1) // 2
                            v_dma(g)
                            if g == 3:
                                todo.extend((-1, fn)
                                            for fn in v_thunks(3)[0:8])
                                todo.extend(
                                    (192 + tt,
                                     lambda tt=tt: v_step(3, tt))
                                    for tt in range(8, KT))
                            else:
                                todo.extend((-1, fn)
                                            for fn in v_thunks(g))
                    _, _, kt_sb, qt_sb = kq_t[p]
                    v_sb = v_t[grp]
                    for qc in range(2):
                        qs = slice(qc * 512, (qc + 1) * 512)
                        ps_e = ps_o.tile([P, 512], F32, tag="o")
                        ps_d = ps_o.tile([P, 512], F32, tag="o")

                        def mk_sc(ki):
                            sc = ps_sc.tile([P, 1024], F32, tag="sc")
                            ks = slice((ki % 4) * P, (ki % 4) * P + P)
                            nc.tensor.matmul(
                                sc[:, 0:512], kt_sb[0:HD, ki // 4, ks],
                                qt_sb[0:HD, qc, :], start=True, stop=True)
                            nc.tensor.matmul(
                                sc[:, 512:1024], kt_sb[HD:P, ki // 4, ks],
                                qt_sb[HD:P, qc, :], start=True, stop=True)
                            return sc

                        def pump(slot, slots_left):
                            n = 0
                            while todo and todo[0][0] != -1 and \
                                    todo[0][0] <= slot:
                                todo.popleft()[1]()
                                n += 1
                            if n == 0 and todo and slots_left > 0 and \
                                    todo[0][0] == -1:
                                want = -(-len(todo) // slots_left)
                                for _ in range(want):
                                    if not todo or todo[0][0] != -1:
                                        break
                                    todo.popleft()[1]()

                        base = (p * 2 + qc) * 16
                        pump(base, 32 - qc * 16)
                        sc_next = mk_sc(0)
                        for ki in range(KT):
                            sc = sc_next
                            if ki + 1 < KT:
                                pump(base + ki + 1,
                                     32 - qc * 16 - ki - 1)
                                sc_next = mk_sc(ki + 1)
                            et = etp.tile([P, 1024], BF, tag="et")
                            nc.scalar.activation(
                                et[:], sc[:], AF.Exp,
                                scale=float(1 / np.sqrt(HD)))
                            if ki == 2 and pending_norm:
                                pending_norm.popleft()()
                            nc.tensor.matmul(
                                ps_e[0:65, :], v_sb[:, ki, pl, 0:65],
                                et[:, 0:512],
                                start=(ki == 0), stop=(ki == KT - 1))
                            nc.tensor.matmul(
                                ps_d[0:65, :], v_sb[:, ki, pl, 65:130],
                                et[:, 512:1024],
                                start=(ki == 0), stop=(ki == KT - 1))
                        # evict unnormalized O (+den row) on ACT engine
                        oe = gbp.tile([65, 512], F32, tag="oe")
                        nc.scalar.activation(oe[:], ps_e[0:65, :], AF.Copy)
                        od = gbp.tile([65, 512], F32, tag="od")
                        nc.scalar.activation(od[:], ps_d[0:65, :], AF.Copy)
                        o_t[(p, qc)] = (oe, od)
                        if p == NPAIR - 1 and qc == 1:
                            while pending_norm:
                                pending_norm.popleft()()
                            emit_norm(p, qc)
                        else:
                            pending_norm.append(
                                lambda p=p, qc=qc: emit_norm(p, qc))
                while pending_norm:
                    pending_norm.popleft()()
                while todo:
                    todo.popleft()[1]()

            if phases == "att":
                nc.sync.dma_start(outT[0:P, 0:512],
                                  osb_big.bitcast(F32)[:, 0, 0:512])
                return

            # ======== P2: out-proj + residual + LN1 ========
            # Residual written in place over x_sb's own-token columns;
            # "s" below is x_sb[:, d, 0:1024].
            def ln1_finish_d(qc, d, mean_b, rstd_b):
                qs = slice(qc * 512, (qc + 1) * 512)
                nc.vector.tensor_sub(x_sb[:, d, qs], x_sb[:, d, qs],
                                     mean_b[:])
                if f["ln1"]:
                    tmp = finp.tile([P, 512], F32, tag="ftmp")
                    nc.vector.tensor_mul(tmp[:], x_sb[:, d, qs],
                                         rstd_b[:])
                    nc.vector.tensor_scalar(
                        x_sb[:, d, qs], tmp[:], g1_sb[:, d:d + 1],
                        be1_sb[:, d:d + 1], ALU.mult, ALU.add)
                else:
                    nc.vector.tensor_mul(x_sb[:, d, qs],
                                         x_sb[:, d, qs], rstd_b[:])

            chain1 = {}
            p2 = contextlib.ExitStack()
            with p2:
                wop = p2.enter_context(tc.tile_pool(name="wop", bufs=8))
                sqp = p2.enter_context(tc.tile_pool(name="sqp", bufs=2))
                ps_ac = p2.enter_context(
                    tc.tile_pool(name="ps_ac", bufs=3, space="PSUM"))
                ps_st = p2.enter_context(
                    tc.tile_pool(name="ps_st", bufs=2, space="PSUM"))

                wo_t = []
                for d in range(DT):
                    wt = wop.tile([P, DT, P], BF, tag="wo")
                    dma_engs[d % 3].dma_start(wt[:], Wo[d])
                    wo_t.append(wt)

                sq_t = {}
                for qc in range(2):
                    qs = slice(qc * 512, (qc + 1) * 512)
                    st_s = ps_st.tile([1, 512], F32, tag="st")
                    st_q = ps_st.tile([1, 512], F32, tag="st")

                    def st_mm(d, qs=qs, st_s=st_s, st_q=st_q):
                        nc.tensor.matmul(st_s[:], ones_cb[:],
                                         x_sb[:, d, qs],
                                         start=(d == 0),
                                         stop=(d == DT - 1))
                        nc.tensor.matmul(st_q[:], ones_cf[:], sq_t[d][:],
                                         start=(d == 0),
                                         stop=(d == DT - 1))

                    for d in range(DT):
                        ps = ps_ac.tile([P, 512], F32, tag="ac")
                        for pr in range(NPAIR):
                            nc.tensor.matmul(ps[:], wo_t[d][:, pr, :],
                                             osb_big[:, pr, qs],
                                             start=(pr == 0),
                                             stop=(pr == NPAIR - 1))
                        if f["bo"]:
                            tmp = finp.tile([P, 512], F32, tag="ftmp")
                            nc.vector.tensor_scalar_add(
                                tmp[:], ps[:], bo_sb[:, d:d + 1])
                            nc.vector.tensor_add(
                                x_sb[:, d, qs], tmp[:], x_sb[:, d, qs])
                        else:
                            nc.vector.tensor_add(x_sb[:, d, qs], ps[:],
                                                 x_sb[:, d, qs])
                        sq = sqp.tile([P, 512], FR, tag="sq")
                        nc.scalar.square(sq[:], x_sb[:, d, qs])
                        sq_t[d] = sq
                        if d >= 1:
                            st_mm(d - 1)
                        if qc == 1 and d >= 1:
                            ln1_finish_d(0, d - 1, *chain1[0])
                    st_mm(DT - 1)
                    chain1[qc] = ln_chain(st_s, st_q)
                    if qc == 1:
                        ln1_finish_d(0, DT - 1, *chain1[0])

            if phases == "p2":
                nc.sync.dma_start(outT[0:P, 0:512],
                                  x_sb.bitcast(F32)[:, 0, 0:512])
                return

            # ======== P3: FFN (x_sb[:, :, 0:1024] holds h) ========
            p3 = contextlib.ExitStack()
            with p3:
                w1p = p3.enter_context(tc.tile_pool(name="w1p", bufs=8))
                ffp = p3.enter_context(tc.tile_pool(name="ffp", bufs=2))
                sqp3 = p3.enter_context(tc.tile_pool(name="sqp3", bufs=2))
                w2p = p3.enter_context(tc.tile_pool(name="w2p", bufs=4))
                ps_ac3 = p3.enter_context(
                    tc.tile_pool(name="ps_ac3", bufs=3, space="PSUM"))
                ps_st3 = p3.enter_context(
                    tc.tile_pool(name="ps_st3", bufs=2, space="PSUM"))
                ps_bc = p3.enter_context(
                    tc.tile_pool(name="ps_bc", bufs=2, space="PSUM"))
                NQ = 4
                FQ = FT // NQ
                # s2 reuses osb_big's pool slot (osb dead after out-proj)
                s2 = obp.tile([P, DT, T_MY], FR, tag="ob", name="s2")

                def ffn1_step(fo, fo_l, qc, w1t):
                    qs = slice(qc * 512, (qc + 1) * 512)
                    ps = ps_ac3.tile([P, 512], F32, tag="ac")
                    for k in range(DT):
                        nc.tensor.matmul(ps[:], w1t[:, k, :],
                                         x_sb[:, k, qs],
                                         start=(k == 0),
                                         stop=(k == DT - 1))
                    nc.scalar.activation(
                        ff1q[:, fo_l, qs], ps[:], AF.Relu,
                        bias=(b1_sb[:, fo:fo + 1] if f["b1"] else 0.0))

                def ffn2_step(quarter, d, qc, w2t):
                    qs = slice(qc * 512, (qc + 1) * 512)
                    ps = ps_ac3.tile([P, 512], F32, tag="ac")
                    for k in range(FQ):
                        nc.tensor.matmul(ps[:], w2t[:, k, :],
                                         ff1q[:, k, qs],
                                         start=(k == 0),
                                         stop=(k == FQ - 1))
                    if quarter == 0:
                        if f["b2"]:
                            nc.vector.tensor_scalar_add(
                                s2[:, d, qs], ps[:], b2_sb[:, d:d + 1])
                            nc.vector.tensor_add(
                                s2[:, d, qs], s2[:, d, qs],
                                x_sb[:, d, qs])
                        else:
                            nc.vector.tensor_add(s2[:, d, qs], ps[:],
                                                 x_sb[:, d, qs])
                    else:
                        nc.vector.tensor_add(s2[:, d, qs],
                                             s2[:, d, qs], ps[:])

                def ln2_finish_d(qc, d, mean_b, rstd_b, eng=None):
                    eng = eng or nc.vector
                    qs = slice(qc * 512, (qc + 1) * 512)
                    eng.tensor_sub(s2[:, d, qs], s2[:, d, qs],
                                   mean_b[:])
                    if f["ln2"]:
                        tmp = finp.tile([P, 512], F32, tag="ftmp")
                        eng.tensor_mul(tmp[:], s2[:, d, qs],
                                       rstd_b[:])
                        eng.tensor_scalar(
                            s2[:, d, qs], tmp[:], g2_sb[:, d:d + 1],
                            be2_sb[:, d:d + 1], ALU.mult, ALU.add)
                    else:
                        eng.tensor_mul(s2[:, d, qs],
                                       s2[:, d, qs], rstd_b[:])
                    nc.sync.dma_start(outT[d * P:(d + 1) * P, qs],
                                      s2[:, d, qs].bitcast(F32))

                for quarter in range(NQ):
                    ff1q = ffp.tile([P, FQ, T_MY], BF, tag="ff1")
                    if quarter == 0:
                        # qc-major: qc0 matmuls overlap LN1-qc1 finish
                        w1ts = []
                        for fo_l in range(FQ):
                            w1t = w1p.tile([P, DT, P], BF, tag="w1")
                            dma_engs[fo_l % 3].dma_start(
                                w1t[:], W1[quarter * FQ + fo_l])
                            w1ts.append(w1t)
                        for qc in range(2):
                            for fo_l in range(FQ):
                                ffn1_step(quarter * FQ + fo_l, fo_l, qc,
                                          w1ts[fo_l])
                                if qc == 0:
                                    ln1_finish_d(1, fo_l, *chain1[1])
                    else:
                        for fo_l in range(FQ):
                            fo = quarter * FQ + fo_l
                            w1t = w1p.tile([P, DT, P], BF, tag="w1")
                            dma_engs[fo_l % 3].dma_start(w1t[:], W1[fo])
                            for qc in range(2):
                                ffn1_step(fo, fo_l, qc, w1t)
                    if quarter < NQ - 1:
                        for d in range(DT):
                            w2t = w2p.tile([P, FQ, P], BF, tag="w2")
                            dma_engs[d % 3].dma_start(w2t[:],
                                                      W2[quarter, d])
                            for qc in range(2):
                                ffn2_step(quarter, d, qc, w2t)
                    else:
                        chain2 = {}
                        for qc in range(2):
                            qs = slice(qc * 512, (qc + 1) * 512)
                            st_s = ps_st3.tile([1, 512], F32, tag="st")
                            st_q = ps_st3.tile([1, 512], F32, tag="st")
                            sq3_t = {}

                            def st3_mm(d, qs=qs, st_s=st_s, st_q=st_q,
                                       sq3_t=sq3_t):
                                nc.tensor.matmul(st_s[:], ones_cf[:],
                                                 s2[:, d, qs],
                                                 start=(d == 0),
                                                 stop=(d == DT - 1))
                                nc.tensor.matmul(st_q[:], ones_cf[:],
                                                 sq3_t[d][:],
                                                 start=(d == 0),
                                                 stop=(d == DT - 1))

                            for d in range(DT):
                                w2t = w2p.tile([P, FQ, P], BF, tag="w2")
                                dma_engs[d % 3].dma_start(
                                    w2t[:], W2[quarter, d])
                                ffn2_step(quarter, d, qc, w2t)
                                sq = sqp3.tile([P, 512], FR, tag="sq3")
                                nc.scalar.square(sq[:], s2[:, d, qs])
                                sq3_t[d] = sq
                                if d >= 1:
                                    st3_mm(d - 1)
                                if qc == 1 and d >= 1:
                                    ln2_finish_d(0, d - 1, *chain2[0])
                            st3_mm(DT - 1)
                            chain2[qc] = ln_chain(
                                st_s, st_q,
                                ps_pool=(ps_bc if qc == 1 else None))
                            if qc == 1:
                                ln2_finish_d(0, DT - 1, *chain2[0])
                        for d in range(DT):
                            ln2_finish_d(1, d, *chain2[1])


# ---------------- host-side helpers ----------------

def shard_inputs(inputs):
    import ml_dtypes
    bf16 = ml_dtypes.bfloat16
    x = np.asarray(inputs["x"], dtype=np.float32)
    shared = {k: np.ascontiguousarray(np.asarray(inputs[k], np.float32))
              for k in ("bq", "bk", "bo", "b1", "b2", "g1", "be1",
                        "g2", "be2")}
    shared["bv"] = np.ascontiguousarray(
        np.asarray(inputs["bv"], np.float32)).astype(bf16)
    Wq = np.asarray(inputs["Wq"], np.float32)
    Wk = np.asarray(inputs["Wk"], np.float32)
    Wv = np.asarray(inputs["Wv"], np.float32)
    Wo = np.asarray(inputs["Wo"], np.float32)
    W1 = np.asarray(inputs["W1"], np.float32)
    W2 = np.asarray(inputs["W2"], np.float32)
    FQ = FT // 4
    shared["Wq"] = np.ascontiguousarray(
        Wq.reshape(DT, P, NPAIR, P).transpose(2, 1, 0, 3)).astype(bf16)
    shared["Wk"] = np.ascontiguousarray(
        Wk.reshape(DT, P, NPAIR, P).transpose(2, 1, 0, 3)).astype(bf16)
    shared["Wv"] = np.ascontiguousarray(
        Wv.reshape(DT, P, 4, 256).transpose(2, 1, 0, 3)).astype(bf16)
    shared["Wo"] = np.ascontiguousarray(
        Wo.reshape(NPAIR, P, DT, P).transpose(2, 1, 0, 3)).astype(bf16)
    shared["W1"] = np.ascontiguousarray(
        W1.reshape(DT, P, FT, P).transpose(2, 1, 0, 3)).astype(bf16)
    shared["W2"] = np.ascontiguousarray(
        W2.reshape(4, FQ, P, DT, P).transpose(0, 3, 2, 1, 4)).astype(bf16)
    maps = []
    for c in range(N_CORES):
        b, h = c // 2, c % 2
        xTb = x[b].T
        roll = np.concatenate([xTb[:, h * T_MY:], xTb[:, :h * T_MY]], axis=1)
        m = {"xT": np.ascontiguousarray(roll).astype(bf16)}
        m.update(shared)
        maps.append(m)
    return maps


def unshard_output(results):
    out = np.empty((B, S, D), np.float32)
    for c in range(N_CORES):
        b, h = c // 2, c % 2
        out[b, h * T_MY:(h + 1) * T_MY, :] = results[c]["outT"].T
    return out


def flags_from_inputs(inputs):
    def nz(k):
        return bool(np.any(np.asarray(inputs[k])))

    return {
        "bq": nz("bq"), "bk": nz("bk"), "bv": nz("bv"), "bo": nz("bo"),
        "b1": nz("b1"), "b2": nz("b2"),
        "ln1": nz("be1") or not np.allclose(np.asarray(inputs["g1"]), 1.0),
        "ln2": nz("be2") or not np.allclose(np.asarray(inputs["g2"]), 1.0),
    }


# ---------------- SPMD runner ----------------


import time

import jax
from jax.sharding import Mesh, PartitionSpec
from jax.experimental.shard_map import shard_map

import concourse.bass2jax as b2j


class SpmdRunner:
    def __init__(self, nc, n_cores: int):
        b2j.install_neuronx_cc_hook()
        self.nc = nc
        self.n_cores = n_cores

        partition_name = (
            nc.partition_id_tensor.name if nc.partition_id_tensor else None
        )
        in_names, out_names, out_avals, zero_outs = [], [], [], []
        for alloc in nc.m.functions[0].allocations:
            if not isinstance(alloc, mybir.MemoryLocationSet):
                continue
            name = alloc.memorylocations[0].name
            if alloc.kind == "ExternalInput":
                if name != partition_name:
                    in_names.append(name)
            elif alloc.kind == "ExternalOutput":
                shape = tuple(alloc.tensor_shape)
                dtype = mybir.dt.np(alloc.dtype)
                out_names.append(name)
                out_avals.append(jax.core.ShapedArray(shape, dtype))
                zero_outs.append(np.zeros(shape, dtype))
        self.in_names, self.out_names = in_names, out_names
        self.out_avals = out_avals
        n_params, n_outs = len(in_names), len(out_names)
        self.n_params = n_params

        all_in_names = list(in_names) + list(out_names)
        if partition_name is not None:
            all_in_names.append(partition_name)

        def _body(*args):
            operands = list(args)
            if partition_name is not None:
                operands.append(b2j.partition_id_tensor())
            outs = b2j._bass_exec_p.bind(
                *operands,
                out_avals=tuple(out_avals),
                in_names=tuple(all_in_names),
                out_names=tuple(out_names),
                lowering_input_output_aliases=(),
                sim_require_finite=True,
                sim_require_nnan=True,
                nc=nc,
            )
            return tuple(outs)

        devices = jax.devices()[:n_cores]
        self.mesh = Mesh(np.asarray(devices), ("core",))
        in_specs = (PartitionSpec("core"),) * (n_params + n_outs)
        out_specs = (PartitionSpec("core"),) * n_outs
        # No donation: keeps zero-out buffers reusable across repeated calls.
        self.fn = jax.jit(
            shard_map(
                _body,
                mesh=self.mesh,
                in_specs=in_specs,
                out_specs=out_specs,
                check_rep=False,
            ),
            keep_unused=True,
        )
        self.zero_outs = zero_outs
        self._dev_zeros = None

    def put_inputs(self, in_maps: list[dict[str, np.ndarray]]):
        """Concat per-core inputs on axis 0 and move to device once."""
        concat = [
            np.concatenate(
                [np.asarray(in_maps[c][n]) for c in range(self.n_cores)], axis=0
            )
            for n in self.in_names
        ]
        sharding = jax.sharding.NamedSharding(self.mesh, PartitionSpec("core"))
        dev_in = [jax.device_put(a, sharding) for a in concat]
        if self._dev_zeros is None:
            self._dev_zeros = [
                jax.device_put(
                    np.zeros((self.n_cores * z.shape[0], *z.shape[1:]), z.dtype),
                    sharding,
                )
                for z in self.zero_outs
            ]
        return dev_in

    def run(self, dev_in):
        outs = self.fn(*dev_in, *self._dev_zeros)
        jax.block_until_ready(outs)
        return outs

    def run_numpy(self, in_maps):
        dev_in = self.put_inputs(in_maps)
        outs = self.run(dev_in)
        res = []
        for c in range(self.n_cores):
            d = {}
            for i, name in enumerate(self.out_names):
                full = np.asarray(outs[i])
                per = full.reshape(self.n_cores, *self.out_avals[i].shape)
                d[name] = per[c]
            res.append(d)
        return res

    def time_runs(self, dev_in, n=10, warmup=2):
        for _ in range(warmup):
            self.run(dev_in)
        times = []
        for _ in range(n):
            t0 = time.perf_counter()
            self.run(dev_in)
            times.append(time.perf_counter() - t0)
        return times


# ---------------- public entry point ----------------

_CACHE = {}


def _get_runner(flag_key, flags):
    if flag_key not in _CACHE:
        nc = build_encoder(flags)
        _CACHE[flag_key] = SpmdRunner(nc, N_CORES)
    return _CACHE[flag_key]


def kernel(**inputs):
    """Full-input encoder layer on 8 NeuronCores; returns [B, S, D] f32."""
    flags = flags_from_inputs(inputs)
    key = tuple(sorted(flags.items()))
    in_maps = shard_inputs(inputs)
    try:
        runner = _get_runner(key, flags)
        results = runner.run_numpy(in_maps)
    except Exception:
        # Device/mesh hiccup: reset backends and retry once from scratch.
        _CACHE.clear()
        try:
            jax.clear_caches()
        except Exception:
            pass
        try:
            jax.extend.backend.clear_backends()
        except Exception:
            pass
        runner = _get_runner(key, flags)
        results = runner.run_numpy(in_maps)
    return unshard_output(results)
